# revision 1
# baseline (speedup 1.0000x reference)
"""Trainium2 Bass kernel for nn_Attention_53231824666818 (GQA attention block).

Sharding: tensor-parallel over heads across 8 NeuronCores. Core c owns query
heads {2c, 2c+1} and kv head c//4 (kv-head groups stay aligned to cores).
Each core computes a full-shape partial of the output projection (row-sharded
Wo); the host sums the 8 partials.

Device-side layout is fully "transposed": activations live as X^T [hid, seq]
so every matmul contracts over the partition dim with no on-device transposes
of X. Scores are computed as S^T [kpos, qpos], which makes the PV product and
the softmax denominator plain matmuls (ones-matmul broadcast trick) and the
per-query normalization a per-column multiply. RMSNorm + RoPE run in [d, seq]
layout: the rstd broadcast comes from an all-ones matmul, rotate_half uses
partition-offset DVE reads, and the norm weight is a per-partition scalar.

All matmul operands use float32r (full PE rate at moving-dim >= 256,
~1.5e-4 relative rounding), accumulating in fp32 PSUM.
"""

import math

import numpy as np

import concourse.bass as bass
import concourse.tile as tile
from concourse import mybir

# ---------------------------------------------------------------------------
# Problem constants (hardcoded; kernel.py must be self-contained).
B, S, HID = 1, 2048, 2048
NH, NKV, HD = 16, 2, 128
G = NH // NKV
EPS = 1e-6
THETA = 1000000.0
NCORES = 8
HPC = NH // NCORES          # query heads per core (2)
SW = 512                    # seq strip width
NSTRIP = S // SW            # 4
NHT = HID // 128            # hid-dim k-tiles (16)
NST = S // 128              # seq 128-tiles (16)
ISQ = 1.0 / math.sqrt(HD)

F32 = mybir.dt.float32
F32R = mybir.dt.float32r
BF16 = mybir.dt.bfloat16

_ALU = mybir.AluOpType
_ACT = mybir.ActivationFunctionType


# ---------------------------------------------------------------------------
# Wait legalization: this walrus build caps fused sync waits at 1 per
# instruction (2 for event-semaphore ops) and rejects any wait on the
# LDWEIGHTS half of a lowered matmul. Tile can attach several waits to one
# instruction (notably the kernel-tail drain), so after TileContext exit we
# hoist excess waits onto same-engine InstNoOp's placed immediately before
# the owner, which blocks the sequencer identically.
_LW_COUNTER = [0]


def _wait_cap(ins) -> int:
    nm = type(ins).__name__
    if nm == "InstMatmult":
        return 0
    if "EventSem" in nm:
        return 2
    return 1


def legalize_waits(nc):
    for fn in nc.m.functions:
        for bb in fn.blocks:
            out = []
            changed = False
            for ins in bb.instructions:
                si = ins.sync_info
                if si is not None:
                    waits = list(si.on_wait or [])
                    cap = _wait_cap(ins)
                    if len(waits) > cap:
                        changed = True
                        for w in waits[cap:]:
                            _LW_COUNTER[0] += 1
                            nop = mybir.InstNoOp(
                                name=f"I-lw-{_LW_COUNTER[0]}",
                                engine=ins.engine,
                                sync_info=mybir.SyncInfo(on_wait=[w], on_update=[]),
                            )
                            out.append(nop)
                        ins.sync_info = mybir.SyncInfo(
                            on_wait=waits[:cap], on_update=list(si.on_update or [])
                        )
                out.append(ins)
            if changed:
                bb.instructions = out
    return nc


# ---------------------------------------------------------------------------
PHASE_MARKS = []


def _mark(nc, label):
    PHASE_MARKS.append((label, int(nc.get_next_instruction_name().split("-")[1])))


def build_nc(legalize=True):
    PHASE_MARKS.clear()
    nc = bass.Bass()

    xT = nc.dram_tensor("xT", [HID, S], F32R, kind="ExternalInput")
    wq = nc.dram_tensor("wq", [128, NHT * HPC * HD], F32R, kind="ExternalInput")
    wk = nc.dram_tensor("wk", [128, NHT * HD], F32R, kind="ExternalInput")
    wv = nc.dram_tensor("wv", [128, NHT * HD], F32R, kind="ExternalInput")
    wo = nc.dram_tensor("wo", [128, HPC * HID], F32R, kind="ExternalInput")
    cosT = nc.dram_tensor("cosT", [HD, S], F32, kind="ExternalInput")
    sinN = nc.dram_tensor("sinN", [HD, S], F32, kind="ExternalInput")
    wqn = nc.dram_tensor("wqn", [HD, 1], F32, kind="ExternalInput")
    wkn = nc.dram_tensor("wkn", [HD, 1], F32, kind="ExternalInput")
    trimask = nc.dram_tensor("trimask", [128, 128], BF16, kind="ExternalInput")
    onesm = nc.dram_tensor("onesm", [128, 128], F32R, kind="ExternalInput")
    ident = nc.dram_tensor("ident", [128, 128], F32, kind="ExternalInput")
    epsb = nc.dram_tensor("epsb", [HD, 1], F32, kind="ExternalInput")
    out = nc.dram_tensor("out", [S, HID], F32, kind="ExternalOutput")

    with tile.TileContext(nc) as tc:
        with tc.tile_pool(name="persist", bufs=1) as pp, \
             tc.tile_pool(name="xtp", bufs=4) as xp, \
             tc.tile_pool(name="epi", bufs=2) as ep, \
             tc.tile_pool(name="exp", bufs=6) as xep, \
             tc.tile_pool(name="obp", bufs=2) as obp, \
             tc.tile_pool(name="ps_acc", bufs=4, space="PSUM") as ps_acc, \
             tc.tile_pool(name="ps_st", bufs=2, space="PSUM") as ps_st, \
             tc.tile_pool(name="ps_pv", bufs=2, space="PSUM") as ps_pv:

            # ---- resident buffers (DMAs emitted lazily below) -------------
            wq_ts = [pp.tile([128, HPC * HD], F32R, tag=f"wq{h}", name=f"wq{h}")
                     for h in range(NHT)]
            wk_ts = [pp.tile([128, HD], F32R, tag=f"wk{h}", name=f"wk{h}") for h in range(NHT)]
            wv_ts = [pp.tile([128, HD], F32R, tag=f"wv{h}", name=f"wv{h}") for h in range(NHT)]
            wo_ts = [pp.tile([128, HID], F32R, tag=f"wo{i}", name=f"wo{i}") for i in range(HPC)]
            cos_t = pp.tile([HD, S], F32, tag="cos")
            sin_t = pp.tile([HD, S], F32, tag="sin")
            wqn_t = pp.tile([HD, 1], F32, tag="wqn")
            wkn_t = pp.tile([HD, 1], F32, tag="wkn")
            tri_t = pp.tile([128, 128], BF16, tag="tri")
            ones_t = pp.tile([128, 128], F32R, tag="ones")
            id_t = pp.tile([128, 128], F32, tag="ident")
            eps_t = pp.tile([HD, 1], F32, tag="eps")

            qt0 = pp.tile([HD, S], F32R, tag="qt0")
            qt1 = pp.tile([HD, S], F32R, tag="qt1")
            kt_sb = pp.tile([HD, S], F32R, tag="ktb")
            v_sb = pp.tile([128, NST * HD], F32R, tag="vsb")
            ot0 = pp.tile([HD, S], F32R, tag="ot0")
            ot1 = pp.tile([HD, S], F32R, tag="ot1")

            def epi_release(acc):
                """Single fast ACT read of the PSUM acc -> SBUF copy, freeing
                the accumulation bank immediately."""
                qc = ep.tile([128, SW], F32, tag="qc")
                nc.scalar.copy(qc[:], acc[:])
                return qc

            def epi_chain(qc, wnorm, dst, sl):
                """RMSNorm (+weight) + RoPE from the SBUF copy -> dst[:, sl]."""
                sq = ep.tile([128, SW], F32R, tag="sq")
                nc.scalar.activation(sq[:], qc[:], _ACT.Square)
                ssq = ps_st.tile([128, SW], F32, tag="st")
                nc.tensor.matmul(ssq[:], ones_t[:], sq[:], start=True, stop=True)
                sd = ep.tile([128, SW], F32, tag="sd")
                nc.scalar.activation(sd[:], ssq[:], _ACT.Sqrt,
                                     scale=1.0 / HD, bias=eps_t[:])
                rstd = ep.tile([128, SW], F32, tag="rstd")
                nc.vector.reciprocal(rstd[:], sd[:])
                qn = ep.tile([128, SW], F32, tag="qn")
                nc.vector.scalar_tensor_tensor(
                    out=qn[:], in0=qc[:], scalar=wnorm[:], in1=rstd[:],
                    op0=_ALU.mult, op1=_ALU.mult)
                t1 = ep.tile([128, SW], F32, tag="t1")
                nc.vector.tensor_tensor(out=t1[:], in0=qn[:], in1=cos_t[:, sl],
                                        op=_ALU.mult)
                u = ep.tile([128, SW], F32, tag="u")
                nc.vector.tensor_tensor(out=u[0:64, :], in0=qn[64:128, :],
                                        in1=sin_t[64:128, sl], op=_ALU.mult)
                nc.vector.tensor_tensor(out=u[64:128, :], in0=qn[0:64, :],
                                        in1=sin_t[0:64, sl], op=_ALU.mult)
                nc.vector.tensor_tensor(out=dst[:, sl], in0=t1[:], in1=u[:],
                                        op=_ALU.add)

            def attention(qt, ot, s):
                """One (head, strip) flash unit: S^T -> exp -> PV^T + denom."""
                sl = bass.ts(s, SW)
                pv = ps_pv.tile([128, SW], F32, tag="pv")
                den = ps_pv.tile([128, SW], F32, tag="pv")
                nk = 4 * s + 4
                for kt in range(nk):
                    st = ps_st.tile([128, SW], F32, tag="st")
                    nc.tensor.matmul(st[:], kt_sb[:, bass.ts(kt, 128)],
                                     qt[:, sl], start=True, stop=True)
                    ex = xep.tile([128, SW], F32R, tag="ex")
                    off = kt - 4 * s
                    if off < 0:
                        nc.scalar.activation(ex[:], st[:], _ACT.Exp, scale=ISQ)
                    else:
                        vs = 128 * off
                        if vs:
                            nc.vector.tensor_scalar_mul(ex[:, 0:vs],
                                                        st[:, 0:vs], 0.0)
                        nc.scalar.activation(ex[:, vs:SW], st[:, vs:SW],
                                             _ACT.Exp, scale=ISQ)
                        nc.vector.tensor_tensor(
                            out=ex[:, vs:vs + 128], in0=ex[:, vs:vs + 128],
                            in1=tri_t[:], op=_ALU.mult)
                    nc.tensor.matmul(pv[:], v_sb[:, bass.ts(kt, 128)], ex[:],
                                     start=(kt == 0), stop=(kt == nk - 1))
                    nc.tensor.matmul(den[:], ones_t[:], ex[:],
                                     start=(kt == 0), stop=(kt == nk - 1))
                rden = ep.tile([128, SW], F32, tag="rden")
                nc.vector.reciprocal(rden[:], den[:])
                nc.vector.tensor_tensor(out=ot[:, sl], in0=pv[:], in1=rden[:],
                                        op=_ALU.mult)

            for s in range(NSTRIP):
                sl = bass.ts(s, SW)
                _mark(nc, f"A{s}")
                # ---- projections: accumulate Q^T/K^T/V^T over hid tiles --
                acc_q0 = ps_acc.tile([128, SW], F32, tag="acc")
                acc_q1 = ps_acc.tile([128, SW], F32, tag="acc")
                acc_k = ps_acc.tile([128, SW], F32, tag="acc")
                acc_v = ps_acc.tile([128, SW], F32, tag="acc")
                import contextlib
                prio = contextlib.nullcontext()
                with prio:
                    for g in range(NHT // 4):
                        if s == 0:
                            # interleave per-h weight chunks with per-h xt
                            # slices so the very first matmul starts after
                            # ~400KB of DMA instead of ~2.5MB
                            xt_g = xp.tile([128, 4, SW], F32R, tag="xt")
                            for j in range(4):
                                h = 4 * g + j
                                nc.sync.dma_start(wq_ts[h][:],
                                                  wq[:, bass.ts(h, HPC * HD)])
                                nc.sync.dma_start(wk_ts[h][:],
                                                  wk[:, bass.ts(h, HD)])
                                nc.sync.dma_start(wv_ts[h][:],
                                                  wv[:, bass.ts(h, HD)])
                                nc.scalar.dma_start(xt_g[:, j, :],
                                                    xT[bass.ts(h, 128), sl])
                        else:
                            xt_g = xp.tile([128, 4, SW], F32R, tag="xt")
                            nc.scalar.dma_start(
                                xt_g[:],
                                xT[bass.ts(g, 512), sl].rearrange(
                                    "(a p) s -> p a s", p=128))
                        for j in range(4):
                            h = 4 * g + j
                            st_, sp_ = (h == 0), (h == NHT - 1)
                            xt_t = xt_g[:, j, :]
                            nc.tensor.matmul(acc_q0[:], wq_ts[h][:, 0:128],
                                             xt_t, start=st_, stop=sp_)
                            nc.tensor.matmul(acc_q1[:], wq_ts[h][:, 128:256],
                                             xt_t, start=st_, stop=sp_)
                            nc.tensor.matmul(acc_k[:], wk_ts[h][:],
                                             xt_t, start=st_, stop=sp_)
                            nc.tensor.matmul(acc_v[:], wv_ts[h][:],
                                             xt_t, start=st_, stop=sp_)

                if s == 0:
                    for t, d in ((cos_t, cosT), (sin_t, sinN), (wqn_t, wqn),
                                 (wkn_t, wkn), (eps_t, epsb), (ones_t, onesm),
                                 (tri_t, trimask), (id_t, ident)):
                        nc.sync.dma_start(t[:], d[:])

                # ---- norm + rope epilogues -------------------------------
                # pass 1: free all four PSUM accumulation banks fast
                _mark(nc, f"epi{s}")
                qc_0 = epi_release(acc_q0)
                qc_k = epi_release(acc_k)
                vtmp = ep.tile([128, SW], F32, tag="vtmp", bufs=1)
                nc.vector.tensor_copy(vtmp[:], acc_v[:])
                qc_1 = epi_release(acc_q1)
                # pass 2: q0 first (gates B0's off-diagonal pairs), then K/V
                epi_chain(qc_0, wqn_t, qt0, sl)
                epi_chain(qc_k, wkn_t, kt_sb, sl)
                for j in range(4):
                    tr = ps_st.tile([128, 128], F32, tag="st")
                    nc.tensor.transpose(tr[:], vtmp[:, bass.ts(j, 128)], id_t[:])
                    nc.vector.tensor_copy(v_sb[:, bass.ts(4 * s + j, 128)], tr[:])
                epi_chain(qc_1, wqn_t, qt1, sl)

                # ---- output projection (delayed one strip so its matmuls
                # fill the next strip's epilogue-chain latency) -------------
                def phase_c(cs):
                    _mark(nc, f"C{cs}")
                    for m in range(4 * cs, 4 * cs + 4):
                        ob = obp.tile([128, HID], F32, tag="ob")
                        for n in range(4):
                            ou = ps_st.tile([128, SW], F32, tag="st")
                            nc.tensor.matmul(ou[:], ot0[:, bass.ts(m, 128)],
                                             wo_ts[0][:, bass.ts(n, SW)],
                                             start=True, stop=False)
                            nc.tensor.matmul(ou[:], ot1[:, bass.ts(m, 128)],
                                             wo_ts[1][:, bass.ts(n, SW)],
                                             start=False, stop=True)
                            if (m + n) % 2:
                                nc.scalar.copy(ob[:, bass.ts(n, SW)], ou[:])
                            else:
                                nc.vector.tensor_copy(ob[:, bass.ts(n, SW)],
                                                      ou[:])
                        nc.sync.dma_start(out[bass.ts(m, 128), :], ob[:])

                if s > 0:
                    phase_c(s - 1)

                # ---- attention for both heads on this strip --------------
                _mark(nc, f"B0s{s}")
                attention(qt0, ot0, s)
                _mark(nc, f"B1s{s}")
                attention(qt1, ot1, s)

                if s == 0:
                    for i in range(HPC):
                        nc.sync.dma_start(wo_ts[i][:],
                                          wo[:, i * HID:(i + 1) * HID])
                if s == NSTRIP - 1:
                    phase_c(s)

    if legalize:
        legalize_waits(nc)
    return nc


# ---------------------------------------------------------------------------
# Host-side input prep.
def _rope_tables(position_ids: np.ndarray):
    pos = position_ids.reshape(-1).astype(np.float64)  # [S]
    j = np.arange(0, HD, 2, dtype=np.float64)
    inv_freq = 1.0 / (THETA ** (j / HD))               # [HD/2]
    freqs = np.outer(inv_freq, pos)                    # [HD/2, S]
    cos_h = np.cos(freqs)
    sin_h = np.sin(freqs)
    cosT = np.concatenate([cos_h, cos_h], axis=0).astype(np.float32)
    sinN = np.concatenate([sin_h, -sin_h], axis=0).astype(np.float32)
    return np.ascontiguousarray(cosT), np.ascontiguousarray(sinN)


def _prep_in_maps(hidden_states, Wq, Wk, Wv, Wo, q_norm_w, k_norm_w,
                  position_ids):
    X = np.asarray(hidden_states, dtype=np.float32).reshape(S, HID)
    xT = np.ascontiguousarray(X.T)
    cosT, sinN = _rope_tables(np.asarray(position_ids))
    wqn = np.ascontiguousarray(
        np.asarray(q_norm_w, dtype=np.float32).reshape(HD, 1))
    wkn = np.ascontiguousarray(
        np.asarray(k_norm_w, dtype=np.float32).reshape(HD, 1))
    import ml_dtypes
    kp, qp = np.meshgrid(np.arange(128), np.arange(128), indexing="ij")
    trimask = (qp >= kp).astype(ml_dtypes.bfloat16)
    onesm = np.ones((128, 128), np.float32)
    ident = np.eye(128, dtype=np.float32)

    Wq = np.asarray(Wq, dtype=np.float32)
    Wk = np.asarray(Wk, dtype=np.float32)
    Wv = np.asarray(Wv, dtype=np.float32)
    Wo = np.asarray(Wo, dtype=np.float32)

    in_maps = []
    for c in range(NCORES):
        kv = c // (NCORES // NKV)
        # [hid, d] -> [128, nht, d] tiled over hid
        wq_c = Wq[:, c * HPC * HD:(c + 1) * HPC * HD]
        wq_l = np.ascontiguousarray(
            wq_c.reshape(NHT, 128, HPC * HD).transpose(1, 0, 2).reshape(
                128, NHT * HPC * HD))
        wk_c = Wk[:, kv * HD:(kv + 1) * HD]
        wk_l = np.ascontiguousarray(
            wk_c.reshape(NHT, 128, HD).transpose(1, 0, 2).reshape(
                128, NHT * HD))
        wv_c = Wv[:, kv * HD:(kv + 1) * HD]
        wv_l = np.ascontiguousarray(
            wv_c.reshape(NHT, 128, HD).transpose(1, 0, 2).reshape(
                128, NHT * HD))
        # Wo rows for this core's two heads: [2*HD, HID] -> [128, 2*HID]
        wo_c = Wo[c * HPC * HD:(c + 1) * HPC * HD, :]
        wo_l = np.ascontiguousarray(
            wo_c.reshape(HPC, HD, HID).transpose(1, 0, 2).reshape(
                128, HPC * HID))
        in_maps.append({
            "xT": xT, "wq": wq_l, "wk": wk_l, "wv": wv_l, "wo": wo_l,
            "cosT": cosT, "sinN": sinN, "wqn": wqn, "wkn": wkn,
            "trimask": trimask, "onesm": onesm, "ident": ident,
            "epsb": np.full((HD, 1), EPS, np.float32),
        })
    return in_maps


# ---------------------------------------------------------------------------
# Runner: persistent jitted shard_map over 8 cores (no donation so device
# buffers are reusable across timing iterations).
_CACHE: dict = {}


def _make_runner(nc):
    import jax
    from jax.sharding import Mesh, PartitionSpec
    try:
        from jax.experimental.shard_map import shard_map
    except ImportError:
        from jax.shard_map import shard_map
    from concourse.bass2jax import (_bass_exec_p, install_neuronx_cc_hook,
                                    partition_id_tensor)

    install_neuronx_cc_hook()

    partition_name = (nc.partition_id_tensor.name
                      if nc.partition_id_tensor else None)
    in_names, out_names, out_avals, zero_outs = [], [], [], []
    for alloc in nc.m.functions[0].allocations:
        if not isinstance(alloc, mybir.MemoryLocationSet):
            continue
        name = alloc.memorylocations[0].name
        if alloc.kind == "ExternalInput":
            if name != partition_name:
                in_names.append(name)
        elif alloc.kind == "ExternalOutput":
            shape = list(alloc.tensor_shape)
            npdt = mybir.dt.np(alloc.dtype)
            out_names.append(name)
            out_avals.append(jax.core.ShapedArray(shape, npdt))
            zero_outs.append(np.zeros(shape, npdt))

    n_params = len(in_names)
    all_in_names = list(in_names) + list(out_names)
    if partition_name is not None:
        all_in_names.append(partition_name)

    def _body(*args):
        operands = list(args)
        if partition_name is not None:
            operands.append(partition_id_tensor())
        outs = _bass_exec_p.bind(
            *operands,
            out_avals=tuple(out_avals),
            in_names=tuple(all_in_names),
            out_names=tuple(out_names),
            lowering_input_output_aliases=(),
            sim_require_finite=True,
            sim_require_nnan=True,
            nc=nc,
        )
        return tuple(outs)

    devices = jax.devices()[:NCORES]
    mesh = Mesh(np.asarray(devices), ("core",))
    n_outs = len(out_names)
    sharded = jax.jit(
        shard_map(_body, mesh=mesh,
                  in_specs=(PartitionSpec("core"),) * (n_params + n_outs),
                  out_specs=(PartitionSpec("core"),) * n_outs,
                  check_rep=False),
        keep_unused=True,
    )
    return {
        "fn": sharded, "in_names": in_names, "out_names": out_names,
        "out_avals": out_avals, "zero_outs": zero_outs, "jax": jax,
    }


def _get_runner(which="main"):
    key = f"runner_{which}"
    if key not in _CACHE:
        nc = build_nc() if which == "main" else build_null_nc()
        _CACHE[key] = _make_runner(nc)
    return _CACHE[key]


def _device_args(in_maps, which="main"):
    r = _get_runner(which)
    jax = r["jax"]
    concat_in = [
        np.concatenate([np.asarray(in_maps[c][name]) for c in range(NCORES)],
                       axis=0)
        for name in r["in_names"]
    ]
    concat_zeros = [
        np.zeros((NCORES * z.shape[0], *z.shape[1:]), z.dtype)
        for z in r["zero_outs"]
    ]
    return [jax.device_put(a) for a in (concat_in + concat_zeros)]


def _run(dargs, which="main"):
    r = _get_runner(which)
    outs = r["fn"](*dargs)
    return outs


def kernel(**inputs) -> np.ndarray:
    in_maps = _prep_in_maps(**inputs)
    dargs = _device_args(in_maps)
    outs = _run(dargs)
    out_c = np.asarray(outs[0]).reshape(NCORES, S, HID)
    full = out_c.sum(axis=0, dtype=np.float64).astype(np.float32)
    return full.reshape(B, S, HID)


def build_null_nc(legalize=True):
    """Input-identical null kernel: same ExternalInput/Output set, but only a
    trivial copy. Used to calibrate away per-dispatch input-staging overhead
    when estimating device execution time."""
    nc = bass.Bass()
    tensors = [
        ("xT", [HID, S], F32R), ("wq", [128, NHT * HPC * HD], F32R),
        ("wk", [128, NHT * HD], F32R), ("wv", [128, NHT * HD], F32R),
        ("wo", [128, HPC * HID], F32R), ("cosT", [HD, S], F32),
        ("sinN", [HD, S], F32), ("wqn", [HD, 1], F32), ("wkn", [HD, 1], F32),
        ("trimask", [128, 128], BF16), ("onesm", [128, 128], F32R),
        ("ident", [128, 128], F32), ("epsb", [HD, 1], F32),
    ]
    handles = {}
    for name, shape, dt in tensors:
        handles[name] = nc.dram_tensor(name, shape, dt, kind="ExternalInput")
    out = nc.dram_tensor("out", [S, HID], F32, kind="ExternalOutput")
    with tile.TileContext(nc) as tc:
        with tc.tile_pool(name="sb", bufs=1) as sb:
            t = sb.tile([128, 128], F32)
            nc.sync.dma_start(t[:], handles["ident"][:])
            nc.sync.dma_start(out[0:128, 0:128], t[:])
    if legalize:
        legalize_waits(nc)
    return nc


def timed_run(inputs, iters=60):
    """Estimate on-device execution time.

    Per-call wall time through the axon tunnel is dominated by input staging
    (~30 ms for this input set), so we interleave single calls of the real
    kernel and an input-identical null kernel and difference the medians of
    the paired per-call times."""
    import time
    in_maps = _prep_in_maps(**inputs)
    d_main = _device_args(in_maps, "main")
    d_null = _device_args(in_maps, "null")
    r_main = _get_runner("main")
    r_null = _get_runner("null")
    jax = r_main["jax"]
    jax.block_until_ready(_run(d_main, "main"))
    jax.block_until_ready(_run(d_null, "null"))

    tm, tn = [], []
    for _ in range(iters):
        t0 = time.perf_counter()
        jax.block_until_ready(_run(d_null, "null"))
        tn.append(time.perf_counter() - t0)
        t0 = time.perf_counter()
        jax.block_until_ready(_run(d_main, "main"))
        tm.append(time.perf_counter() - t0)
    tm, tn = np.array(tm), np.array(tn)
    est = float(np.median(tm) - np.median(tn))
    return max(est, 0.0), float(np.median(tm)), float(np.median(tn))



# revision 5
# speedup vs baseline: 11.3643x; 11.3643x over previous
"""Trainium2 Bass kernel for nn_Attention_53231824666818 (GQA attention block).

Sharding: tensor-parallel over heads across 8 NeuronCores. Core c owns query
heads {2c, 2c+1} and kv head c//4 (kv-head groups stay aligned to cores).
Each core computes a full-shape partial of the output projection (row-sharded
Wo); the host sums the 8 partials (fp16 partials, fp32 accumulation).

Device-side layout is fully "transposed": activations live as X^T [hid, seq]
so every matmul contracts over the partition dim with no on-device transposes
of X. Scores are computed as S^T [kpos, qpos], which makes the PV product and
the softmax denominator plain matmuls (ones-matmul broadcast trick) and the
per-query normalization a per-column multiply.

Perf structure (v2):
- All matmul operands are fp16: full PE rate, FWL fast weight loads (fp32r
  gets neither), half the HBM traffic of fp32.
- The only ACT functions used are Exp/Ln/Square/Copy - all in the single
  `natural_log_exp_and_others` table set, so no ACT table reloads.
  RMSNorm rstd = exp(-0.5*ln(ms)); softmax 1/den = exp(-ln(den)). This also
  avoids nc.vector.reciprocal (~4us per [128,512] on DVE).
- Whole X^T is loaded to SBUF up front (8.4 MB fp16) in 4 strip-sized DMAs;
  weights stream in parallel on the other HWDGE ring.
- Three phases, each keeping PE busy back-to-back so the HAM clock gate
  stays at 2.4 GHz: (1) projections + norm/rope epilogues (epilogue chains
  trail one projection target behind PE), (2) flash attention with a
  depth-2 software pipeline (scores matmul for tile t+2 issued before PV of
  tile t, hiding the ACT exp latency), (3) output projection.
- exp uses bias=-EXPC so fp16 probs cannot overflow (softmax shift
  invariance makes this exact).
- V is accumulated directly in [seq, hd] layout (stationary = X^T chunk)
  so no PE transposes are needed.
"""

import math

import numpy as np

import concourse.bass as bass
import concourse.tile as tile
from concourse import mybir

# ---------------------------------------------------------------------------
# Problem constants (hardcoded; kernel.py must be self-contained).
B, S, HID = 1, 2048, 2048
NH, NKV, HD = 16, 2, 128
G = NH // NKV
EPS = 1e-6
THETA = 1000000.0
NCORES = 8
HPC = NH // NCORES          # query heads per core (2)
SW = 512                    # seq strip width
NSTRIP = S // SW            # 4
NHT = HID // 128            # hid-dim k-tiles (16)
NST = S // 128              # seq 128-tiles (16)
ISQ = 1.0 / math.sqrt(HD)
EXPC = 2.0                  # exp bias: keeps fp16 probs < 65504

F32 = mybir.dt.float32
FP16 = mybir.dt.float16

_ALU = mybir.AluOpType
_ACT = mybir.ActivationFunctionType


# ---------------------------------------------------------------------------
# Wait legalization: this walrus build caps fused sync waits at 1 per
# instruction (2 for event-semaphore ops) and rejects any wait on the
# LDWEIGHTS half of a lowered matmul. Tile can attach several waits to one
# instruction (notably the kernel-tail drain), so after TileContext exit we
# hoist excess waits onto same-engine InstNoOp's placed immediately before
# the owner, which blocks the sequencer identically.
_LW_COUNTER = [0]


def _wait_cap(ins) -> int:
    nm = type(ins).__name__
    if nm == "InstMatmult":
        return 0
    if "EventSem" in nm:
        return 2
    return 1


def legalize_waits(nc):
    for fn in nc.m.functions:
        for bb in fn.blocks:
            out = []
            changed = False
            for ins in bb.instructions:
                si = ins.sync_info
                if si is not None:
                    waits = list(si.on_wait or [])
                    cap = _wait_cap(ins)
                    if len(waits) > cap:
                        changed = True
                        for w in waits[cap:]:
                            _LW_COUNTER[0] += 1
                            nop = mybir.InstNoOp(
                                name=f"I-lw-{_LW_COUNTER[0]}",
                                engine=ins.engine,
                                sync_info=mybir.SyncInfo(on_wait=[w], on_update=[]),
                            )
                            out.append(nop)
                        ins.sync_info = mybir.SyncInfo(
                            on_wait=waits[:cap], on_update=list(si.on_update or [])
                        )
                out.append(ins)
            if changed:
                bb.instructions = out
    return nc


# ---------------------------------------------------------------------------
PHASE_MARKS = []


def _mark(nc, label):
    PHASE_MARKS.append((label, int(nc.get_next_instruction_name().split("-")[1])))


def build_nc(legalize=True):
    PHASE_MARKS.clear()
    nc = bass.Bass()

    xtb = nc.dram_tensor("xtb", [128, NSTRIP * NHT * SW], FP16, kind="ExternalInput")
    wqb = nc.dram_tensor("wqb", [128, NHT * HPC * HD], FP16, kind="ExternalInput")
    wkb = nc.dram_tensor("wkb", [128, NHT * HD], FP16, kind="ExternalInput")
    wvb = nc.dram_tensor("wvb", [128, NHT * HD], FP16, kind="ExternalInput")
    wob = nc.dram_tensor("wob", [128, HPC * HID], FP16, kind="ExternalInput")
    cosT = nc.dram_tensor("cosT", [HD, S], F32, kind="ExternalInput")
    sinN = nc.dram_tensor("sinN", [HD, S], F32, kind="ExternalInput")
    wqn = nc.dram_tensor("wqn", [HD, 1], F32, kind="ExternalInput")
    wkn = nc.dram_tensor("wkn", [HD, 1], F32, kind="ExternalInput")
    trimask = nc.dram_tensor("trimask", [128, 128], FP16, kind="ExternalInput")
    onesm = nc.dram_tensor("onesm", [128, 128], FP16, kind="ExternalInput")
    epsb = nc.dram_tensor("epsb", [HD, 1], F32, kind="ExternalInput")
    expcb = nc.dram_tensor("expcb", [HD, 1], F32, kind="ExternalInput")
    out = nc.dram_tensor("out", [S, HID], FP16, kind="ExternalOutput")

    with tile.TileContext(nc) as tc:
        with tc.tile_pool(name="persist", bufs=1) as pp, \
             tc.tile_pool(name="epi", bufs=2) as ep, \
             tc.tile_pool(name="exp", bufs=6) as xep, \
             tc.tile_pool(name="obp", bufs=2) as obp, \
             tc.tile_pool(name="ps_acc", bufs=2, space="PSUM") as pacc, \
             tc.tile_pool(name="ps_st", bufs=3, space="PSUM") as pst, \
             tc.tile_pool(name="ps_pv", bufs=3, space="PSUM") as ppv:

            # ---- persistent SBUF residents --------------------------------
            xt_ts = [pp.tile([128, NHT, SW], FP16, tag=f"xt{s}", name=f"xt{s}")
                     for s in range(NSTRIP)]
            wq_t = pp.tile([128, NHT, HPC * HD], FP16, tag="wq")
            wk_t = pp.tile([128, NHT, HD], FP16, tag="wk")
            wv_t = pp.tile([128, NHT, HD], FP16, tag="wv")
            wo_t = pp.tile([128, HPC, HID], FP16, tag="wo")
            cos_t = pp.tile([HD, S], F32, tag="cos")
            sin_t = pp.tile([HD, S], F32, tag="sin")
            wqn_t = pp.tile([HD, 1], F32, tag="wqn")
            wkn_t = pp.tile([HD, 1], F32, tag="wkn")
            eps_t = pp.tile([HD, 1], F32, tag="eps")
            exc_t = pp.tile([HD, 1], F32, tag="exc")
            tri_t = pp.tile([128, 128], FP16, tag="tri")
            ones_t = pp.tile([128, 128], FP16, tag="ones")

            qt0 = pp.tile([HD, S], FP16, tag="qt0")
            qt1 = pp.tile([HD, S], FP16, tag="qt1")
            kt_sb = pp.tile([HD, S], FP16, tag="ktb")
            v_sb = pp.tile([128, NST * HD], FP16, tag="vsb")
            ot0 = pp.tile([HD, S], FP16, tag="ot0")
            ot1 = pp.tile([HD, S], FP16, tag="ot1")

            # ---- input DMAs: X strips on the ACT HWDGE ring, weights and
            # tables on the sync ring, in first-use order ---------------------
            for s in range(NSTRIP):
                nc.scalar.dma_start(xt_ts[s][:], xtb[:, bass.ts(s, NHT * SW)])
            nc.sync.dma_start(wq_t[:], wqb[:])
            nc.sync.dma_start(wk_t[:], wkb[:])
            nc.sync.dma_start(wv_t[:], wvb[:])
            nc.sync.dma_start(cos_t[:], cosT[:])
            nc.sync.dma_start(sin_t[:], sinN[:])
            for t, d in ((wqn_t, wqn), (wkn_t, wkn), (eps_t, epsb),
                         (exc_t, expcb), (tri_t, trimask), (ones_t, onesm)):
                nc.sync.dma_start(t[:], d[:])
            nc.sync.dma_start(wo_t[:], wob[:])

            # ---- norm + rope epilogue, split in two so the ssq matmul never
            # stalls PE: partA (engine-only drains of the PSUM acc) is emitted
            # right after the projection; chainB one projection target later.
            def epi_partA(acc):
                sq = ep.tile([128, SW], FP16, tag="sq")
                nc.scalar.activation(sq[:], acc[:], _ACT.Square)
                qc = ep.tile([128, SW], F32, tag="qc")
                nc.vector.tensor_copy(qc[:], acc[:])
                return sq, qc

            def epi_chainB(sq, qc, wnorm, dst, sl):
                ssq = pst.tile([128, SW], F32, tag="st")
                nc.tensor.matmul(ssq[:], ones_t[:], sq[:], start=True, stop=True)
                # rstd = (ms)^-1/2 = exp(-0.5*ln(ssq/HD + eps)); Ln/Exp live in
                # one ACT table set (sqrt does not, and DVE reciprocal is slow)
                lnv = ep.tile([128, SW], F32, tag="lnv")
                nc.scalar.activation(lnv[:], ssq[:], _ACT.Ln,
                                     scale=1.0 / HD, bias=eps_t[:])
                rstd = ep.tile([128, SW], F32, tag="rstd")
                nc.scalar.activation(rstd[:], lnv[:], _ACT.Exp, scale=-0.5)
                qn = ep.tile([128, SW], F32, tag="qn")
                nc.vector.scalar_tensor_tensor(
                    out=qn[:], in0=qc[:], scalar=wnorm[:], in1=rstd[:],
                    op0=_ALU.mult, op1=_ALU.mult)
                t1 = ep.tile([128, SW], F32, tag="t1")
                nc.vector.tensor_tensor(out=t1[:], in0=qn[:], in1=cos_t[:, sl],
                                        op=_ALU.mult)
                u = ep.tile([128, SW], F32, tag="u")
                nc.vector.tensor_tensor(out=u[0:64, :], in0=qn[64:128, :],
                                        in1=sin_t[64:128, sl], op=_ALU.mult)
                nc.vector.tensor_tensor(out=u[64:128, :], in0=qn[0:64, :],
                                        in1=sin_t[0:64, sl], op=_ALU.mult)
                nc.vector.tensor_tensor(out=dst[:, sl], in0=t1[:], in1=u[:],
                                        op=_ALU.add)

            # ---- phase 1: projections ------------------------------------
            pending = None
            for s in range(NSTRIP):
                sl = bass.ts(s, SW)
                _mark(nc, f"A{s}")
                xts = xt_ts[s]
                targets = (
                    (lambda h: wq_t[:, h, 0:HD], wqn_t, qt0),
                    (lambda h: wq_t[:, h, HD:2 * HD], wqn_t, qt1),
                    (lambda h: wk_t[:, h, :], wkn_t, kt_sb),
                )
                for wsel, wnorm, dst in targets:
                    acc = pacc.tile([128, SW], F32, tag="acc")
                    for h in range(NHT):
                        nc.tensor.matmul(acc[:], wsel(h), xts[:, h, :],
                                         start=(h == 0), stop=(h == NHT - 1))
                    pa = epi_partA(acc)
                    if pending is not None:
                        epi_chainB(*pending)
                    pending = (pa[0], pa[1], wnorm, dst, sl)
                # V: accumulate directly in [seq, hd] (stationary = X^T chunk)
                for j in range(4):
                    vacc = pacc.tile([128, HD], F32, tag="acc")
                    for h in range(NHT):
                        nc.tensor.matmul(vacc[:], xts[:, h, bass.ts(j, 128)],
                                         wv_t[:, h, :],
                                         start=(h == 0), stop=(h == NHT - 1))
                    nc.vector.tensor_copy(v_sb[:, bass.ts(4 * s + j, 128)],
                                          vacc[:])
            epi_chainB(*pending)

            # ---- phase 2: attention (depth-2 software pipeline) -----------
            def attention(qt, ot, s):
                sl = bass.ts(s, SW)
                nk = 4 * s + 4
                pv = ppv.tile([128, SW], F32, tag="pv")
                den = ppv.tile([128, SW], F32, tag="pv")
                exs = {}

                def emit_st_exp(t):
                    off = t - 4 * s
                    vs = 0 if off < 0 else 128 * off
                    st = pst.tile([128, SW], F32, tag="st")
                    nc.tensor.matmul(st[:, vs:], kt_sb[:, bass.ts(t, 128)],
                                     qt[:, SW * s + vs:SW * (s + 1)],
                                     start=True, stop=True)
                    ex = xep.tile([128, SW], FP16, tag="ex")
                    nc.scalar.activation(ex[:, vs:], st[:, vs:], _ACT.Exp,
                                         scale=ISQ, bias=exc_t[:])
                    if off >= 0:
                        # diagonal 128x128 block needs the triangular mask
                        nc.vector.tensor_tensor(
                            out=ex[:, vs:vs + 128], in0=ex[:, vs:vs + 128],
                            in1=tri_t[:], op=_ALU.mult)
                    exs[t] = (ex, vs)

                emit_st_exp(0)
                if nk > 1:
                    emit_st_exp(1)
                for t in range(nk):
                    if t + 2 < nk:
                        emit_st_exp(t + 2)
                    ex, vs = exs.pop(t)
                    st_, sp_ = (t == 0), (t == nk - 1)
                    nc.tensor.matmul(pv[:, vs:], v_sb[:, bass.ts(t, 128)],
                                     ex[:, vs:], start=st_, stop=sp_)
                    nc.tensor.matmul(den[:, vs:], ones_t[:], ex[:, vs:],
                                     start=st_, stop=sp_)
                # 1/den = exp(-ln(den)) on ACT (same table set as the exps)
                lnd = ep.tile([128, SW], F32, tag="lnd")
                nc.scalar.activation(lnd[:], den[:], _ACT.Ln)
                rdn = ep.tile([128, SW], F32, tag="rdn")
                nc.scalar.activation(rdn[:], lnd[:], _ACT.Exp, scale=-1.0)
                nc.vector.tensor_tensor(out=ot[:, sl], in0=pv[:], in1=rdn[:],
                                        op=_ALU.mult)

            # ---- phase 3: output projection (per 128-row tile) ------------
            def outproj(cs):
                _mark(nc, f"C{cs}")
                for m in range(4 * cs, 4 * cs + 4):
                    ob = obp.tile([128, HID], FP16, tag="ob")
                    for n in range(4):
                        ou = pst.tile([128, SW], F32, tag="st")
                        nc.tensor.matmul(ou[:], ot0[:, bass.ts(m, 128)],
                                         wo_t[:, 0, bass.ts(n, SW)],
                                         start=True, stop=False)
                        nc.tensor.matmul(ou[:], ot1[:, bass.ts(m, 128)],
                                         wo_t[:, 1, bass.ts(n, SW)],
                                         start=False, stop=True)
                        if (m + n) % 2:
                            nc.scalar.copy(ob[:, bass.ts(n, SW)], ou[:])
                        else:
                            nc.vector.tensor_copy(ob[:, bass.ts(n, SW)], ou[:])
                    nc.sync.dma_start(out[bass.ts(m, 128), :], ob[:])

            for s in range(NSTRIP):
                _mark(nc, f"B0s{s}")
                attention(qt0, ot0, s)
                _mark(nc, f"B1s{s}")
                attention(qt1, ot1, s)
                if s > 0:
                    outproj(s - 1)
            outproj(NSTRIP - 1)

    if legalize:
        legalize_waits(nc)
    return nc


# ---------------------------------------------------------------------------
# Host-side input prep.
def _rope_tables(position_ids: np.ndarray):
    pos = position_ids.reshape(-1).astype(np.float64)  # [S]
    j = np.arange(0, HD, 2, dtype=np.float64)
    inv_freq = 1.0 / (THETA ** (j / HD))               # [HD/2]
    freqs = np.outer(inv_freq, pos)                    # [HD/2, S]
    cos_h = np.cos(freqs)
    sin_h = np.sin(freqs)
    cosT = np.concatenate([cos_h, cos_h], axis=0).astype(np.float32)
    sinN = np.concatenate([sin_h, -sin_h], axis=0).astype(np.float32)
    return np.ascontiguousarray(cosT), np.ascontiguousarray(sinN)


def _prep_in_maps(hidden_states, Wq, Wk, Wv, Wo, q_norm_w, k_norm_w,
                  position_ids):
    X = np.asarray(hidden_states, dtype=np.float32).reshape(S, HID)
    # xtb[p, s, h, w] = X[s*SW+w, h*128+p]
    xtb = np.ascontiguousarray(
        X.reshape(NSTRIP, SW, NHT, 128).transpose(3, 0, 2, 1)
        .reshape(128, NSTRIP * NHT * SW)).astype(np.float16)
    cosT, sinN = _rope_tables(np.asarray(position_ids))
    wqn = np.ascontiguousarray(
        np.asarray(q_norm_w, dtype=np.float32).reshape(HD, 1))
    wkn = np.ascontiguousarray(
        np.asarray(k_norm_w, dtype=np.float32).reshape(HD, 1))
    kp, qp = np.meshgrid(np.arange(128), np.arange(128), indexing="ij")
    trimask = (qp >= kp).astype(np.float16)
    onesm = np.ones((128, 128), np.float16)

    Wq = np.asarray(Wq, dtype=np.float32)
    Wk = np.asarray(Wk, dtype=np.float32)
    Wv = np.asarray(Wv, dtype=np.float32)
    Wo = np.asarray(Wo, dtype=np.float32)

    in_maps = []
    for c in range(NCORES):
        kv = c // (NCORES // NKV)
        wq_c = Wq[:, c * HPC * HD:(c + 1) * HPC * HD]
        wq_l = np.ascontiguousarray(
            wq_c.reshape(NHT, 128, HPC * HD).transpose(1, 0, 2).reshape(
                128, NHT * HPC * HD)).astype(np.float16)
        wk_c = Wk[:, kv * HD:(kv + 1) * HD]
        wk_l = np.ascontiguousarray(
            wk_c.reshape(NHT, 128, HD).transpose(1, 0, 2).reshape(
                128, NHT * HD)).astype(np.float16)
        wv_c = Wv[:, kv * HD:(kv + 1) * HD]
        wv_l = np.ascontiguousarray(
            wv_c.reshape(NHT, 128, HD).transpose(1, 0, 2).reshape(
                128, NHT * HD)).astype(np.float16)
        wo_c = Wo[c * HPC * HD:(c + 1) * HPC * HD, :]
        wo_l = np.ascontiguousarray(
            wo_c.reshape(HPC, HD, HID).transpose(1, 0, 2).reshape(
                128, HPC * HID)).astype(np.float16)
        in_maps.append({
            "xtb": xtb, "wqb": wq_l, "wkb": wk_l, "wvb": wv_l, "wob": wo_l,
            "cosT": cosT, "sinN": sinN, "wqn": wqn, "wkn": wkn,
            "trimask": trimask, "onesm": onesm,
            "epsb": np.full((HD, 1), EPS, np.float32),
            "expcb": np.full((HD, 1), -EXPC, np.float32),
        })
    return in_maps


# ---------------------------------------------------------------------------
# Runner: persistent jitted shard_map over 8 cores (no donation so device
# buffers are reusable across timing iterations).
_CACHE: dict = {}


def _make_runner(nc):
    import jax
    from jax.sharding import Mesh, PartitionSpec
    try:
        from jax.experimental.shard_map import shard_map
    except ImportError:
        from jax.shard_map import shard_map
    from concourse.bass2jax import (_bass_exec_p, install_neuronx_cc_hook,
                                    partition_id_tensor)

    install_neuronx_cc_hook()

    partition_name = (nc.partition_id_tensor.name
                      if nc.partition_id_tensor else None)
    in_names, out_names, out_avals, zero_outs = [], [], [], []
    for alloc in nc.m.functions[0].allocations:
        if not isinstance(alloc, mybir.MemoryLocationSet):
            continue
        name = alloc.memorylocations[0].name
        if alloc.kind == "ExternalInput":
            if name != partition_name:
                in_names.append(name)
        elif alloc.kind == "ExternalOutput":
            shape = list(alloc.tensor_shape)
            npdt = mybir.dt.np(alloc.dtype)
            out_names.append(name)
            out_avals.append(jax.core.ShapedArray(shape, npdt))
            zero_outs.append(np.zeros(shape, npdt))

    n_params = len(in_names)
    all_in_names = list(in_names) + list(out_names)
    if partition_name is not None:
        all_in_names.append(partition_name)

    def _body(*args):
        operands = list(args)
        if partition_name is not None:
            operands.append(partition_id_tensor())
        outs = _bass_exec_p.bind(
            *operands,
            out_avals=tuple(out_avals),
            in_names=tuple(all_in_names),
            out_names=tuple(out_names),
            lowering_input_output_aliases=(),
            sim_require_finite=True,
            sim_require_nnan=True,
            nc=nc,
        )
        return tuple(outs)

    devices = jax.devices()[:NCORES]
    mesh = Mesh(np.asarray(devices), ("core",))
    n_outs = len(out_names)
    sharded = jax.jit(
        shard_map(_body, mesh=mesh,
                  in_specs=(PartitionSpec("core"),) * (n_params + n_outs),
                  out_specs=(PartitionSpec("core"),) * n_outs,
                  check_rep=False),
        keep_unused=True,
    )
    return {
        "fn": sharded, "in_names": in_names, "out_names": out_names,
        "out_avals": out_avals, "zero_outs": zero_outs, "jax": jax,
    }


def _get_runner(which="main"):
    key = f"runner_{which}"
    if key not in _CACHE:
        nc = build_nc() if which == "main" else build_null_nc()
        _CACHE[f"nc_{which}"] = nc
        _CACHE[key] = _make_runner(nc)
    return _CACHE[key]


def _device_args(in_maps, which="main"):
    r = _get_runner(which)
    jax = r["jax"]
    concat_in = [
        np.concatenate([np.asarray(in_maps[c][name]) for c in range(NCORES)],
                       axis=0)
        for name in r["in_names"]
    ]
    concat_zeros = [
        np.zeros((NCORES * z.shape[0], *z.shape[1:]), z.dtype)
        for z in r["zero_outs"]
    ]
    return [jax.device_put(a) for a in (concat_in + concat_zeros)]


def _run(dargs, which="main"):
    r = _get_runner(which)
    outs = r["fn"](*dargs)
    return outs


def kernel(**inputs) -> np.ndarray:
    in_maps = _prep_in_maps(**inputs)
    dargs = _device_args(in_maps)
    outs = _run(dargs)
    out_c = np.asarray(outs[0]).reshape(NCORES, S, HID)
    full = out_c.astype(np.float32).sum(axis=0).astype(np.float32)
    return full.reshape(B, S, HID)


def build_null_nc(legalize=True):
    """Input-identical null kernel: same ExternalInput/Output set, but only a
    trivial copy. Used to calibrate away per-dispatch input-staging overhead
    when estimating device execution time."""
    nc = bass.Bass()
    tensors = [
        ("xtb", [128, NSTRIP * NHT * SW], FP16),
        ("wqb", [128, NHT * HPC * HD], FP16),
        ("wkb", [128, NHT * HD], FP16), ("wvb", [128, NHT * HD], FP16),
        ("wob", [128, HPC * HID], FP16), ("cosT", [HD, S], F32),
        ("sinN", [HD, S], F32), ("wqn", [HD, 1], F32), ("wkn", [HD, 1], F32),
        ("trimask", [128, 128], FP16), ("onesm", [128, 128], FP16),
        ("epsb", [HD, 1], F32), ("expcb", [HD, 1], F32),
    ]
    handles = {}
    for name, shape, dt in tensors:
        handles[name] = nc.dram_tensor(name, shape, dt, kind="ExternalInput")
    out = nc.dram_tensor("out", [S, HID], FP16, kind="ExternalOutput")
    with tile.TileContext(nc) as tc:
        with tc.tile_pool(name="sb", bufs=1) as sb:
            t = sb.tile([128, 128], FP16)
            nc.sync.dma_start(t[:], handles["trimask"][:])
            nc.sync.dma_start(out[0:128, 0:128], t[:])
    if legalize:
        legalize_waits(nc)
    return nc


def timed_run(inputs, iters=60):
    """Estimate on-device execution time.

    Per-call wall time through the axon tunnel is dominated by input staging
    (~30 ms for this input set), so we interleave single calls of the real
    kernel and an input-identical null kernel and difference the medians of
    the paired per-call times."""
    import time
    in_maps = _prep_in_maps(**inputs)
    d_main = _device_args(in_maps, "main")
    d_null = _device_args(in_maps, "null")
    r_main = _get_runner("main")
    r_null = _get_runner("null")
    jax = r_main["jax"]
    jax.block_until_ready(_run(d_main, "main"))
    jax.block_until_ready(_run(d_null, "null"))

    tm, tn = [], []
    for _ in range(iters):
        t0 = time.perf_counter()
        jax.block_until_ready(_run(d_null, "null"))
        tn.append(time.perf_counter() - t0)
        t0 = time.perf_counter()
        jax.block_until_ready(_run(d_main, "main"))
        tm.append(time.perf_counter() - t0)
    tm, tn = np.array(tm), np.array(tn)
    est = float(np.median(tm) - np.median(tn))
    return max(est, 0.0), float(np.median(tm)), float(np.median(tn))


# revision 6
# speedup vs baseline: 11.9423x; 1.0509x over previous
"""Trainium2 Bass kernel for nn_Attention_53231824666818 (GQA attention block).

Sharding: tensor-parallel over heads across 8 NeuronCores. Core c owns query
heads {2c, 2c+1} and kv head c//4 (kv-head groups stay aligned to cores).
Each core computes a full-shape partial of the output projection (row-sharded
Wo); the host sums the 8 partials (fp16 partials, fp32 accumulation).

Device-side layout is fully "transposed": activations live as X^T [hid, seq]
so every matmul contracts over the partition dim with no on-device transposes
of X. Scores are computed as S^T [kpos, qpos], which makes the PV product and
the softmax denominator plain matmuls (ones-matmul broadcast trick) and the
per-query normalization a per-column multiply.

Perf structure (v2):
- All matmul operands are fp16: full PE rate, FWL fast weight loads (fp32r
  gets neither), half the HBM traffic of fp32.
- The only ACT functions used are Exp/Ln/Square/Copy - all in the single
  `natural_log_exp_and_others` table set, so no ACT table reloads.
  RMSNorm rstd = exp(-0.5*ln(ms)); softmax 1/den = exp(-ln(den)). This also
  avoids nc.vector.reciprocal (~4us per [128,512] on DVE).
- Whole X^T is loaded to SBUF up front (8.4 MB fp16) in 4 strip-sized DMAs;
  weights stream in parallel on the other HWDGE ring.
- Three phases, each keeping PE busy back-to-back so the HAM clock gate
  stays at 2.4 GHz: (1) projections + norm/rope epilogues (epilogue chains
  trail one projection target behind PE), (2) flash attention with a
  depth-2 software pipeline (scores matmul for tile t+2 issued before PV of
  tile t, hiding the ACT exp latency), (3) output projection.
- exp uses bias=-EXPC so fp16 probs cannot overflow (softmax shift
  invariance makes this exact).
- V is accumulated directly in [seq, hd] layout (stationary = X^T chunk)
  so no PE transposes are needed.
"""

import math

import numpy as np

import concourse.bass as bass
import concourse.tile as tile
from concourse import mybir

# ---------------------------------------------------------------------------
# Problem constants (hardcoded; kernel.py must be self-contained).
B, S, HID = 1, 2048, 2048
NH, NKV, HD = 16, 2, 128
G = NH // NKV
EPS = 1e-6
THETA = 1000000.0
NCORES = 8
HPC = NH // NCORES          # query heads per core (2)
SW = 512                    # seq strip width
NSTRIP = S // SW            # 4
NHT = HID // 128            # hid-dim k-tiles (16)
NST = S // 128              # seq 128-tiles (16)
ISQ = 1.0 / math.sqrt(HD)
EXPC = 2.0                  # exp bias: keeps fp16 probs < 65504

F32 = mybir.dt.float32
FP16 = mybir.dt.float16

_ALU = mybir.AluOpType
_ACT = mybir.ActivationFunctionType


# ---------------------------------------------------------------------------
# Wait legalization: this walrus build caps fused sync waits at 1 per
# instruction (2 for event-semaphore ops) and rejects any wait on the
# LDWEIGHTS half of a lowered matmul. Tile can attach several waits to one
# instruction (notably the kernel-tail drain), so after TileContext exit we
# hoist excess waits onto same-engine InstNoOp's placed immediately before
# the owner, which blocks the sequencer identically.
_LW_COUNTER = [0]


def _wait_cap(ins) -> int:
    nm = type(ins).__name__
    if nm == "InstMatmult":
        return 0
    if "EventSem" in nm:
        return 2
    return 1


def legalize_waits(nc):
    for fn in nc.m.functions:
        for bb in fn.blocks:
            out = []
            changed = False
            for ins in bb.instructions:
                si = ins.sync_info
                if si is not None:
                    waits = list(si.on_wait or [])
                    cap = _wait_cap(ins)
                    if len(waits) > cap:
                        changed = True
                        for w in waits[cap:]:
                            _LW_COUNTER[0] += 1
                            nop = mybir.InstNoOp(
                                name=f"I-lw-{_LW_COUNTER[0]}",
                                engine=ins.engine,
                                sync_info=mybir.SyncInfo(on_wait=[w], on_update=[]),
                            )
                            out.append(nop)
                        ins.sync_info = mybir.SyncInfo(
                            on_wait=waits[:cap], on_update=list(si.on_update or [])
                        )
                out.append(ins)
            if changed:
                bb.instructions = out
    return nc


# ---------------------------------------------------------------------------
PHASE_MARKS = []


def _mark(nc, label):
    PHASE_MARKS.append((label, int(nc.get_next_instruction_name().split("-")[1])))


def build_nc(legalize=True):
    PHASE_MARKS.clear()
    nc = bass.Bass()

    xtb = nc.dram_tensor("xtb", [128, NSTRIP * NHT * SW], FP16, kind="ExternalInput")
    wqb = nc.dram_tensor("wqb", [128, NHT * HPC * HD], FP16, kind="ExternalInput")
    wkb = nc.dram_tensor("wkb", [128, NHT * HD], FP16, kind="ExternalInput")
    wvb = nc.dram_tensor("wvb", [128, NHT * HD], FP16, kind="ExternalInput")
    wob = nc.dram_tensor("wob", [128, HPC * HID], FP16, kind="ExternalInput")
    cosT = nc.dram_tensor("cosT", [HD, S], F32, kind="ExternalInput")
    sinN = nc.dram_tensor("sinN", [HD, S], F32, kind="ExternalInput")
    wqn = nc.dram_tensor("wqn", [HD, 1], F32, kind="ExternalInput")
    wkn = nc.dram_tensor("wkn", [HD, 1], F32, kind="ExternalInput")
    trimask = nc.dram_tensor("trimask", [128, 128], FP16, kind="ExternalInput")
    onesm = nc.dram_tensor("onesm", [128, 128], FP16, kind="ExternalInput")
    epsb = nc.dram_tensor("epsb", [HD, 1], F32, kind="ExternalInput")
    expcb = nc.dram_tensor("expcb", [HD, 1], F32, kind="ExternalInput")
    out = nc.dram_tensor("out", [S, HID], FP16, kind="ExternalOutput")

    with tile.TileContext(nc) as tc:
        with tc.tile_pool(name="persist", bufs=1) as pp, \
             tc.tile_pool(name="epi", bufs=2) as ep, \
             tc.tile_pool(name="exp", bufs=6) as xep, \
             tc.tile_pool(name="obp", bufs=2) as obp, \
             tc.tile_pool(name="ps_acc", bufs=2, space="PSUM") as pacc, \
             tc.tile_pool(name="ps_st", bufs=3, space="PSUM") as pst, \
             tc.tile_pool(name="ps_pv", bufs=3, space="PSUM") as ppv:

            # ---- persistent SBUF residents --------------------------------
            # X strips split into half-tiles so the first projection matmuls
            # gate on a 1 MB DMA instead of the full 2.1 MB strip
            xt_ts = [(pp.tile([128, NHT // 2, SW], FP16, tag=f"xt{s}a", name=f"xt{s}a"),
                      pp.tile([128, NHT // 2, SW], FP16, tag=f"xt{s}b", name=f"xt{s}b"))
                     for s in range(NSTRIP)]
            wq_t = pp.tile([128, NHT, HPC * HD], FP16, tag="wq")
            wk_t = pp.tile([128, NHT, HD], FP16, tag="wk")
            wv_t = pp.tile([128, NHT, HD], FP16, tag="wv")
            wo_t = pp.tile([128, HPC, HID], FP16, tag="wo")
            cos_t = pp.tile([HD, S], F32, tag="cos")
            sin_t = pp.tile([HD, S], F32, tag="sin")
            wqn_t = pp.tile([HD, 1], F32, tag="wqn")
            wkn_t = pp.tile([HD, 1], F32, tag="wkn")
            eps_t = pp.tile([HD, 1], F32, tag="eps")
            exc_t = pp.tile([HD, 1], F32, tag="exc")
            tri_t = pp.tile([128, 128], FP16, tag="tri")
            ones_t = pp.tile([128, 128], FP16, tag="ones")

            qt0 = pp.tile([HD, S], FP16, tag="qt0")
            qt1 = pp.tile([HD, S], FP16, tag="qt1")
            kt_sb = pp.tile([HD, S], FP16, tag="ktb")
            v_sb = pp.tile([128, NST * HD], FP16, tag="vsb")
            ot0 = pp.tile([HD, S], FP16, tag="ot0")
            ot1 = pp.tile([HD, S], FP16, tag="ot1")

            # ---- input DMAs: ALL on the sync HWDGE ring, in first-use
            # order. Never issue big DMAs from a compute engine: Tile's 8
            # DMAHW completion lanes recycle, so a trigger can block its
            # issuing engine's whole queue behind an earlier DMA (measured
            # 35 us of ACT stall when X strips were triggered from ACT).
            HSW = (NHT // 2) * SW
            nc.sync.dma_start(wq_t[:], wqb[:])
            nc.sync.dma_start(xt_ts[0][0][:], xtb[:, 0:HSW])
            nc.sync.dma_start(xt_ts[0][1][:], xtb[:, HSW:2 * HSW])
            nc.sync.dma_start(wk_t[:], wkb[:])
            nc.sync.dma_start(wv_t[:], wvb[:])
            nc.sync.dma_start(xt_ts[1][0][:], xtb[:, 2 * HSW:3 * HSW])
            nc.sync.dma_start(xt_ts[1][1][:], xtb[:, 3 * HSW:4 * HSW])
            nc.sync.dma_start(cos_t[:], cosT[:])
            nc.sync.dma_start(sin_t[:], sinN[:])
            for t, d in ((wqn_t, wqn), (wkn_t, wkn), (eps_t, epsb),
                         (exc_t, expcb), (tri_t, trimask), (ones_t, onesm)):
                nc.sync.dma_start(t[:], d[:])
            for s in (2, 3):
                nc.sync.dma_start(xt_ts[s][0][:], xtb[:, 2 * s * HSW:(2 * s + 1) * HSW])
                nc.sync.dma_start(xt_ts[s][1][:], xtb[:, (2 * s + 1) * HSW:(2 * s + 2) * HSW])
            nc.sync.dma_start(wo_t[:], wob[:])

            # ---- norm + rope epilogue, split in two so the ssq matmul never
            # stalls PE: partA (engine-only drains of the PSUM acc) is emitted
            # right after the projection; chainB one projection target later.
            def epi_partA(acc):
                sq = ep.tile([128, SW], FP16, tag="sq")
                nc.scalar.activation(sq[:], acc[:], _ACT.Square)
                qc = ep.tile([128, SW], F32, tag="qc")
                nc.vector.tensor_copy(qc[:], acc[:])
                return sq, qc

            def epi_chainB(sq, qc, wnorm, dst, sl):
                ssq = pst.tile([128, SW], F32, tag="st")
                nc.tensor.matmul(ssq[:], ones_t[:], sq[:], start=True, stop=True)
                # rstd = (ms)^-1/2 = exp(-0.5*ln(ssq/HD + eps)); Ln/Exp live in
                # one ACT table set (sqrt does not, and DVE reciprocal is slow)
                lnv = ep.tile([128, SW], F32, tag="lnv")
                nc.scalar.activation(lnv[:], ssq[:], _ACT.Ln,
                                     scale=1.0 / HD, bias=eps_t[:])
                rstd = ep.tile([128, SW], F32, tag="rstd")
                nc.scalar.activation(rstd[:], lnv[:], _ACT.Exp, scale=-0.5)
                qn = ep.tile([128, SW], F32, tag="qn")
                nc.vector.scalar_tensor_tensor(
                    out=qn[:], in0=qc[:], scalar=wnorm[:], in1=rstd[:],
                    op0=_ALU.mult, op1=_ALU.mult)
                t1 = ep.tile([128, SW], F32, tag="t1")
                nc.vector.tensor_tensor(out=t1[:], in0=qn[:], in1=cos_t[:, sl],
                                        op=_ALU.mult)
                u = ep.tile([128, SW], F32, tag="u")
                nc.vector.tensor_tensor(out=u[0:64, :], in0=qn[64:128, :],
                                        in1=sin_t[64:128, sl], op=_ALU.mult)
                nc.vector.tensor_tensor(out=u[64:128, :], in0=qn[0:64, :],
                                        in1=sin_t[0:64, sl], op=_ALU.mult)
                nc.vector.tensor_tensor(out=dst[:, sl], in0=t1[:], in1=u[:],
                                        op=_ALU.add)

            # ---- phase 1: projections ------------------------------------
            pending = None
            for s in range(NSTRIP):
                sl = bass.ts(s, SW)
                _mark(nc, f"A{s}")
                xta, xtb_ = xt_ts[s]
                xsel = lambda h: (xta if h < NHT // 2 else xtb_)[:, h % (NHT // 2), :]
                targets = (
                    (lambda h: wq_t[:, h, 0:HD], wqn_t, qt0),
                    (lambda h: wq_t[:, h, HD:2 * HD], wqn_t, qt1),
                    (lambda h: wk_t[:, h, :], wkn_t, kt_sb),
                )
                for wsel, wnorm, dst in targets:
                    acc = pacc.tile([128, SW], F32, tag="acc")
                    for h in range(NHT):
                        nc.tensor.matmul(acc[:], wsel(h), xsel(h),
                                         start=(h == 0), stop=(h == NHT - 1))
                    pa = epi_partA(acc)
                    if pending is not None:
                        epi_chainB(*pending)
                    pending = (pa[0], pa[1], wnorm, dst, sl)
                # V: accumulate directly in [seq, hd] (stationary = X^T chunk)
                for j in range(4):
                    vacc = pacc.tile([128, HD], F32, tag="acc")
                    for h in range(NHT):
                        xh = (xta if h < NHT // 2 else xtb_)
                        nc.tensor.matmul(vacc[:], xh[:, h % (NHT // 2), bass.ts(j, 128)],
                                         wv_t[:, h, :],
                                         start=(h == 0), stop=(h == NHT - 1))
                    nc.vector.tensor_copy(v_sb[:, bass.ts(4 * s + j, 128)],
                                          vacc[:])
            epi_chainB(*pending)

            # ---- phase 2: attention (depth-2 software pipeline) -----------
            def attention(qt, ot, s):
                sl = bass.ts(s, SW)
                nk = 4 * s + 4
                pv = ppv.tile([128, SW], F32, tag="pv")
                den = ppv.tile([128, SW], F32, tag="pv")
                exs = {}

                def emit_st_exp(t):
                    off = t - 4 * s
                    vs = 0 if off < 0 else 128 * off
                    st = pst.tile([128, SW], F32, tag="st")
                    nc.tensor.matmul(st[:, vs:], kt_sb[:, bass.ts(t, 128)],
                                     qt[:, SW * s + vs:SW * (s + 1)],
                                     start=True, stop=True)
                    ex = xep.tile([128, SW], FP16, tag="ex")
                    nc.scalar.activation(ex[:, vs:], st[:, vs:], _ACT.Exp,
                                         scale=ISQ, bias=exc_t[:])
                    if off >= 0:
                        # diagonal 128x128 block needs the triangular mask
                        nc.vector.tensor_tensor(
                            out=ex[:, vs:vs + 128], in0=ex[:, vs:vs + 128],
                            in1=tri_t[:], op=_ALU.mult)
                    exs[t] = (ex, vs)

                emit_st_exp(0)
                if nk > 1:
                    emit_st_exp(1)
                for t in range(nk):
                    if t + 2 < nk:
                        emit_st_exp(t + 2)
                    ex, vs = exs.pop(t)
                    st_, sp_ = (t == 0), (t == nk - 1)
                    nc.tensor.matmul(pv[:, vs:], v_sb[:, bass.ts(t, 128)],
                                     ex[:, vs:], start=st_, stop=sp_)
                    nc.tensor.matmul(den[:, vs:], ones_t[:], ex[:, vs:],
                                     start=st_, stop=sp_)
                # 1/den = exp(-ln(den)) on ACT (same table set as the exps)
                lnd = ep.tile([128, SW], F32, tag="lnd")
                nc.scalar.activation(lnd[:], den[:], _ACT.Ln)
                rdn = ep.tile([128, SW], F32, tag="rdn")
                nc.scalar.activation(rdn[:], lnd[:], _ACT.Exp, scale=-1.0)
                nc.vector.tensor_tensor(out=ot[:, sl], in0=pv[:], in1=rdn[:],
                                        op=_ALU.mult)

            # ---- phase 3: output projection (per 128-row tile) ------------
            def outproj(cs):
                _mark(nc, f"C{cs}")
                for m in range(4 * cs, 4 * cs + 4):
                    ob = obp.tile([128, HID], FP16, tag="ob")
                    for n in range(4):
                        ou = pst.tile([128, SW], F32, tag="st")
                        nc.tensor.matmul(ou[:], ot0[:, bass.ts(m, 128)],
                                         wo_t[:, 0, bass.ts(n, SW)],
                                         start=True, stop=False)
                        nc.tensor.matmul(ou[:], ot1[:, bass.ts(m, 128)],
                                         wo_t[:, 1, bass.ts(n, SW)],
                                         start=False, stop=True)
                        if (m + n) % 2:
                            nc.scalar.copy(ob[:, bass.ts(n, SW)], ou[:])
                        else:
                            nc.vector.tensor_copy(ob[:, bass.ts(n, SW)], ou[:])
                    nc.sync.dma_start(out[bass.ts(m, 128), :], ob[:])

            for s in range(NSTRIP):
                _mark(nc, f"B0s{s}")
                attention(qt0, ot0, s)
                _mark(nc, f"B1s{s}")
                attention(qt1, ot1, s)
                if s > 0:
                    outproj(s - 1)
            outproj(NSTRIP - 1)

    if legalize:
        legalize_waits(nc)
    return nc


# ---------------------------------------------------------------------------
# Host-side input prep.
def _rope_tables(position_ids: np.ndarray):
    pos = position_ids.reshape(-1).astype(np.float64)  # [S]
    j = np.arange(0, HD, 2, dtype=np.float64)
    inv_freq = 1.0 / (THETA ** (j / HD))               # [HD/2]
    freqs = np.outer(inv_freq, pos)                    # [HD/2, S]
    cos_h = np.cos(freqs)
    sin_h = np.sin(freqs)
    cosT = np.concatenate([cos_h, cos_h], axis=0).astype(np.float32)
    sinN = np.concatenate([sin_h, -sin_h], axis=0).astype(np.float32)
    return np.ascontiguousarray(cosT), np.ascontiguousarray(sinN)


def _prep_in_maps(hidden_states, Wq, Wk, Wv, Wo, q_norm_w, k_norm_w,
                  position_ids):
    X = np.asarray(hidden_states, dtype=np.float32).reshape(S, HID)
    # xtb[p, s, h, w] = X[s*SW+w, h*128+p]
    xtb = np.ascontiguousarray(
        X.reshape(NSTRIP, SW, NHT, 128).transpose(3, 0, 2, 1)
        .reshape(128, NSTRIP * NHT * SW)).astype(np.float16)
    cosT, sinN = _rope_tables(np.asarray(position_ids))
    wqn = np.ascontiguousarray(
        np.asarray(q_norm_w, dtype=np.float32).reshape(HD, 1))
    wkn = np.ascontiguousarray(
        np.asarray(k_norm_w, dtype=np.float32).reshape(HD, 1))
    kp, qp = np.meshgrid(np.arange(128), np.arange(128), indexing="ij")
    trimask = (qp >= kp).astype(np.float16)
    onesm = np.ones((128, 128), np.float16)

    Wq = np.asarray(Wq, dtype=np.float32)
    Wk = np.asarray(Wk, dtype=np.float32)
    Wv = np.asarray(Wv, dtype=np.float32)
    Wo = np.asarray(Wo, dtype=np.float32)

    in_maps = []
    for c in range(NCORES):
        kv = c // (NCORES // NKV)
        wq_c = Wq[:, c * HPC * HD:(c + 1) * HPC * HD]
        wq_l = np.ascontiguousarray(
            wq_c.reshape(NHT, 128, HPC * HD).transpose(1, 0, 2).reshape(
                128, NHT * HPC * HD)).astype(np.float16)
        wk_c = Wk[:, kv * HD:(kv + 1) * HD]
        wk_l = np.ascontiguousarray(
            wk_c.reshape(NHT, 128, HD).transpose(1, 0, 2).reshape(
                128, NHT * HD)).astype(np.float16)
        wv_c = Wv[:, kv * HD:(kv + 1) * HD]
        wv_l = np.ascontiguousarray(
            wv_c.reshape(NHT, 128, HD).transpose(1, 0, 2).reshape(
                128, NHT * HD)).astype(np.float16)
        wo_c = Wo[c * HPC * HD:(c + 1) * HPC * HD, :]
        wo_l = np.ascontiguousarray(
            wo_c.reshape(HPC, HD, HID).transpose(1, 0, 2).reshape(
                128, HPC * HID)).astype(np.float16)
        in_maps.append({
            "xtb": xtb, "wqb": wq_l, "wkb": wk_l, "wvb": wv_l, "wob": wo_l,
            "cosT": cosT, "sinN": sinN, "wqn": wqn, "wkn": wkn,
            "trimask": trimask, "onesm": onesm,
            "epsb": np.full((HD, 1), EPS, np.float32),
            "expcb": np.full((HD, 1), -EXPC, np.float32),
        })
    return in_maps


# ---------------------------------------------------------------------------
# Runner: persistent jitted shard_map over 8 cores (no donation so device
# buffers are reusable across timing iterations).
_CACHE: dict = {}


def _make_runner(nc):
    import jax
    from jax.sharding import Mesh, PartitionSpec
    try:
        from jax.experimental.shard_map import shard_map
    except ImportError:
        from jax.shard_map import shard_map
    from concourse.bass2jax import (_bass_exec_p, install_neuronx_cc_hook,
                                    partition_id_tensor)

    install_neuronx_cc_hook()

    partition_name = (nc.partition_id_tensor.name
                      if nc.partition_id_tensor else None)
    in_names, out_names, out_avals, zero_outs = [], [], [], []
    for alloc in nc.m.functions[0].allocations:
        if not isinstance(alloc, mybir.MemoryLocationSet):
            continue
        name = alloc.memorylocations[0].name
        if alloc.kind == "ExternalInput":
            if name != partition_name:
                in_names.append(name)
        elif alloc.kind == "ExternalOutput":
            shape = list(alloc.tensor_shape)
            npdt = mybir.dt.np(alloc.dtype)
            out_names.append(name)
            out_avals.append(jax.core.ShapedArray(shape, npdt))
            zero_outs.append(np.zeros(shape, npdt))

    n_params = len(in_names)
    all_in_names = list(in_names) + list(out_names)
    if partition_name is not None:
        all_in_names.append(partition_name)

    def _body(*args):
        operands = list(args)
        if partition_name is not None:
            operands.append(partition_id_tensor())
        outs = _bass_exec_p.bind(
            *operands,
            out_avals=tuple(out_avals),
            in_names=tuple(all_in_names),
            out_names=tuple(out_names),
            lowering_input_output_aliases=(),
            sim_require_finite=True,
            sim_require_nnan=True,
            nc=nc,
        )
        return tuple(outs)

    devices = jax.devices()[:NCORES]
    mesh = Mesh(np.asarray(devices), ("core",))
    n_outs = len(out_names)
    sharded = jax.jit(
        shard_map(_body, mesh=mesh,
                  in_specs=(PartitionSpec("core"),) * (n_params + n_outs),
                  out_specs=(PartitionSpec("core"),) * n_outs,
                  check_rep=False),
        keep_unused=True,
    )
    return {
        "fn": sharded, "in_names": in_names, "out_names": out_names,
        "out_avals": out_avals, "zero_outs": zero_outs, "jax": jax,
    }


def _get_runner(which="main"):
    key = f"runner_{which}"
    if key not in _CACHE:
        nc = build_nc() if which == "main" else build_null_nc()
        _CACHE[f"nc_{which}"] = nc
        _CACHE[key] = _make_runner(nc)
    return _CACHE[key]


def _device_args(in_maps, which="main"):
    r = _get_runner(which)
    jax = r["jax"]
    concat_in = [
        np.concatenate([np.asarray(in_maps[c][name]) for c in range(NCORES)],
                       axis=0)
        for name in r["in_names"]
    ]
    concat_zeros = [
        np.zeros((NCORES * z.shape[0], *z.shape[1:]), z.dtype)
        for z in r["zero_outs"]
    ]
    return [jax.device_put(a) for a in (concat_in + concat_zeros)]


def _run(dargs, which="main"):
    r = _get_runner(which)
    outs = r["fn"](*dargs)
    return outs


def kernel(**inputs) -> np.ndarray:
    in_maps = _prep_in_maps(**inputs)
    dargs = _device_args(in_maps)
    outs = _run(dargs)
    out_c = np.asarray(outs[0]).reshape(NCORES, S, HID)
    full = out_c.astype(np.float32).sum(axis=0).astype(np.float32)
    return full.reshape(B, S, HID)


def build_null_nc(legalize=True):
    """Input-identical null kernel: same ExternalInput/Output set, but only a
    trivial copy. Used to calibrate away per-dispatch input-staging overhead
    when estimating device execution time."""
    nc = bass.Bass()
    tensors = [
        ("xtb", [128, NSTRIP * NHT * SW], FP16),
        ("wqb", [128, NHT * HPC * HD], FP16),
        ("wkb", [128, NHT * HD], FP16), ("wvb", [128, NHT * HD], FP16),
        ("wob", [128, HPC * HID], FP16), ("cosT", [HD, S], F32),
        ("sinN", [HD, S], F32), ("wqn", [HD, 1], F32), ("wkn", [HD, 1], F32),
        ("trimask", [128, 128], FP16), ("onesm", [128, 128], FP16),
        ("epsb", [HD, 1], F32), ("expcb", [HD, 1], F32),
    ]
    handles = {}
    for name, shape, dt in tensors:
        handles[name] = nc.dram_tensor(name, shape, dt, kind="ExternalInput")
    out = nc.dram_tensor("out", [S, HID], FP16, kind="ExternalOutput")
    with tile.TileContext(nc) as tc:
        with tc.tile_pool(name="sb", bufs=1) as sb:
            t = sb.tile([128, 128], FP16)
            nc.sync.dma_start(t[:], handles["trimask"][:])
            nc.sync.dma_start(out[0:128, 0:128], t[:])
    if legalize:
        legalize_waits(nc)
    return nc


def timed_run(inputs, iters=60):
    """Estimate on-device execution time.

    Per-call wall time through the axon tunnel is dominated by input staging
    (~30 ms for this input set), so we interleave single calls of the real
    kernel and an input-identical null kernel and difference the medians of
    the paired per-call times."""
    import time
    in_maps = _prep_in_maps(**inputs)
    d_main = _device_args(in_maps, "main")
    d_null = _device_args(in_maps, "null")
    r_main = _get_runner("main")
    r_null = _get_runner("null")
    jax = r_main["jax"]
    jax.block_until_ready(_run(d_main, "main"))
    jax.block_until_ready(_run(d_null, "null"))

    tm, tn = [], []
    for _ in range(iters):
        t0 = time.perf_counter()
        jax.block_until_ready(_run(d_null, "null"))
        tn.append(time.perf_counter() - t0)
        t0 = time.perf_counter()
        jax.block_until_ready(_run(d_main, "main"))
        tm.append(time.perf_counter() - t0)
    tm, tn = np.array(tm), np.array(tn)
    est = float(np.median(tm) - np.median(tn))
    return max(est, 0.0), float(np.median(tm)), float(np.median(tn))


# revision 8
# speedup vs baseline: 12.2384x; 1.0248x over previous
"""Trainium2 Bass kernel for nn_Attention_53231824666818 (GQA attention block).

Sharding: tensor-parallel over heads across 8 NeuronCores. Core c owns query
heads {2c, 2c+1} and kv head c//4 (kv-head groups stay aligned to cores).
Each core computes a full-shape partial of the output projection (row-sharded
Wo); the host sums the 8 partials (fp16 partials, fp32 accumulation).

Device-side layout is fully "transposed": activations live as X^T [hid, seq]
so every matmul contracts over the partition dim with no on-device transposes
of X. Scores are computed as S^T [kpos, qpos], which makes the PV product and
the softmax denominator plain matmuls (ones-matmul broadcast trick) and the
per-query normalization a per-column multiply.

Perf structure (v2):
- All matmul operands are fp16: full PE rate, FWL fast weight loads (fp32r
  gets neither), half the HBM traffic of fp32.
- The only ACT functions used are Exp/Ln/Square/Copy - all in the single
  `natural_log_exp_and_others` table set, so no ACT table reloads.
  RMSNorm rstd = exp(-0.5*ln(ms)); softmax 1/den = exp(-ln(den)). This also
  avoids nc.vector.reciprocal (~4us per [128,512] on DVE).
- Whole X^T is loaded to SBUF up front (8.4 MB fp16) in 4 strip-sized DMAs;
  weights stream in parallel on the other HWDGE ring.
- Three phases, each keeping PE busy back-to-back so the HAM clock gate
  stays at 2.4 GHz: (1) projections + norm/rope epilogues (epilogue chains
  trail one projection target behind PE), (2) flash attention with a
  depth-2 software pipeline (scores matmul for tile t+2 issued before PV of
  tile t, hiding the ACT exp latency), (3) output projection.
- exp uses bias=-EXPC so fp16 probs cannot overflow (softmax shift
  invariance makes this exact).
- V is accumulated directly in [seq, hd] layout (stationary = X^T chunk)
  so no PE transposes are needed.
"""

import math

import numpy as np

import concourse.bass as bass
import concourse.tile as tile
from concourse import mybir

# ---------------------------------------------------------------------------
# Problem constants (hardcoded; kernel.py must be self-contained).
B, S, HID = 1, 2048, 2048
NH, NKV, HD = 16, 2, 128
G = NH // NKV
EPS = 1e-6
THETA = 1000000.0
NCORES = 8
HPC = NH // NCORES          # query heads per core (2)
SW = 512                    # seq strip width
NSTRIP = S // SW            # 4
NHT = HID // 128            # hid-dim k-tiles (16)
NST = S // 128              # seq 128-tiles (16)
ISQ = 1.0 / math.sqrt(HD)
EXPC = 2.0                  # exp bias: keeps fp16 probs < 65504

F32 = mybir.dt.float32
FP16 = mybir.dt.float16

_ALU = mybir.AluOpType
_ACT = mybir.ActivationFunctionType


# ---------------------------------------------------------------------------
# Wait legalization: this walrus build caps fused sync waits at 1 per
# instruction (2 for event-semaphore ops) and rejects any wait on the
# LDWEIGHTS half of a lowered matmul. Tile can attach several waits to one
# instruction (notably the kernel-tail drain), so after TileContext exit we
# hoist excess waits onto same-engine InstNoOp's placed immediately before
# the owner, which blocks the sequencer identically.
_LW_COUNTER = [0]


def _wait_cap(ins) -> int:
    nm = type(ins).__name__
    if nm == "InstMatmult":
        return 0
    if "EventSem" in nm:
        return 2
    return 1


def legalize_waits(nc):
    for fn in nc.m.functions:
        for bb in fn.blocks:
            out = []
            changed = False
            for ins in bb.instructions:
                si = ins.sync_info
                if si is not None:
                    waits = list(si.on_wait or [])
                    cap = _wait_cap(ins)
                    if len(waits) > cap:
                        changed = True
                        for w in waits[cap:]:
                            _LW_COUNTER[0] += 1
                            nop = mybir.InstNoOp(
                                name=f"I-lw-{_LW_COUNTER[0]}",
                                engine=ins.engine,
                                sync_info=mybir.SyncInfo(on_wait=[w], on_update=[]),
                            )
                            out.append(nop)
                        ins.sync_info = mybir.SyncInfo(
                            on_wait=waits[:cap], on_update=list(si.on_update or [])
                        )
                out.append(ins)
            if changed:
                bb.instructions = out
    return nc


# ---------------------------------------------------------------------------
PHASE_MARKS = []


def _mark(nc, label):
    PHASE_MARKS.append((label, int(nc.get_next_instruction_name().split("-")[1])))


def build_nc(legalize=True):
    PHASE_MARKS.clear()
    nc = bass.Bass()

    xtb = nc.dram_tensor("xtb", [128, NSTRIP * NHT * SW], FP16, kind="ExternalInput")
    wqb = nc.dram_tensor("wqb", [128, NHT * HPC * HD], FP16, kind="ExternalInput")
    wkb = nc.dram_tensor("wkb", [128, NHT * HD], FP16, kind="ExternalInput")
    wvb = nc.dram_tensor("wvb", [128, NHT * HD], FP16, kind="ExternalInput")
    wob = nc.dram_tensor("wob", [128, HPC * HID], FP16, kind="ExternalInput")
    cosT = nc.dram_tensor("cosT", [HD, S], F32, kind="ExternalInput")
    sinN = nc.dram_tensor("sinN", [HD, S], F32, kind="ExternalInput")
    wqn = nc.dram_tensor("wqn", [HD, 1], F32, kind="ExternalInput")
    wkn = nc.dram_tensor("wkn", [HD, 1], F32, kind="ExternalInput")
    trimask = nc.dram_tensor("trimask", [128, 128], FP16, kind="ExternalInput")
    onesm = nc.dram_tensor("onesm", [128, 128], FP16, kind="ExternalInput")
    epsb = nc.dram_tensor("epsb", [HD, 1], F32, kind="ExternalInput")
    expcb = nc.dram_tensor("expcb", [HD, 1], F32, kind="ExternalInput")
    out = nc.dram_tensor("out", [S, HID], FP16, kind="ExternalOutput")

    with tile.TileContext(nc) as tc:
        with tc.tile_pool(name="persist", bufs=1) as pp, \
             tc.tile_pool(name="epi", bufs=2) as ep, \
             tc.tile_pool(name="exp", bufs=6) as xep, \
             tc.tile_pool(name="obp", bufs=2) as obp, \
             tc.tile_pool(name="ps_acc", bufs=2, space="PSUM") as pacc, \
             tc.tile_pool(name="ps_st", bufs=2, space="PSUM") as pst, \
             tc.tile_pool(name="ps_pv", bufs=2, space="PSUM") as ppv:

            # ---- persistent SBUF residents --------------------------------
            # X strips split into half-tiles so the first projection matmuls
            # gate on a 1 MB DMA instead of the full 2.1 MB strip
            xt_ts = [(pp.tile([128, NHT // 2, SW], FP16, tag=f"xt{s}a", name=f"xt{s}a"),
                      pp.tile([128, NHT // 2, SW], FP16, tag=f"xt{s}b", name=f"xt{s}b"))
                     for s in range(NSTRIP)]
            wq_t = pp.tile([128, NHT, HPC * HD], FP16, tag="wq")
            wk_t = pp.tile([128, NHT, HD], FP16, tag="wk")
            wv_t = pp.tile([128, NHT, HD], FP16, tag="wv")
            wo_t = pp.tile([128, HPC, HID], FP16, tag="wo")
            cos_t = pp.tile([HD, S], F32, tag="cos")
            sin_t = pp.tile([HD, S], F32, tag="sin")
            wqn_t = pp.tile([HD, 1], F32, tag="wqn")
            wkn_t = pp.tile([HD, 1], F32, tag="wkn")
            eps_t = pp.tile([HD, 1], F32, tag="eps")
            exc_t = pp.tile([HD, 1], F32, tag="exc")
            tri_t = pp.tile([128, 128], FP16, tag="tri")
            ones_t = pp.tile([128, 128], FP16, tag="ones")

            qt0 = pp.tile([HD, S], FP16, tag="qt0")
            qt1 = pp.tile([HD, S], FP16, tag="qt1")
            kt_sb = pp.tile([HD, S], FP16, tag="ktb")
            v_sb = pp.tile([128, NST * HD], FP16, tag="vsb")
            ot0 = pp.tile([HD, S], FP16, tag="ot0")
            ot1 = pp.tile([HD, S], FP16, tag="ot1")

            # ---- input DMAs: ALL on the sync HWDGE ring, in first-use
            # order. Never issue big DMAs from a compute engine: Tile's 8
            # DMAHW completion lanes recycle, so a trigger can block its
            # issuing engine's whole queue behind an earlier DMA (measured
            # 35 us of ACT stall when X strips were triggered from ACT).
            HSW = (NHT // 2) * SW
            nc.sync.dma_start(xt_ts[0][0][:], xtb[:, 0:HSW])
            nc.sync.dma_start(wq_t[:], wqb[:])
            nc.sync.dma_start(xt_ts[0][1][:], xtb[:, HSW:2 * HSW])
            nc.sync.dma_start(wk_t[:], wkb[:])
            nc.sync.dma_start(wv_t[:], wvb[:])
            nc.sync.dma_start(xt_ts[1][0][:], xtb[:, 2 * HSW:3 * HSW])
            nc.sync.dma_start(xt_ts[1][1][:], xtb[:, 3 * HSW:4 * HSW])
            nc.sync.dma_start(cos_t[:], cosT[:])
            nc.sync.dma_start(sin_t[:], sinN[:])
            for t, d in ((wqn_t, wqn), (wkn_t, wkn), (eps_t, epsb),
                         (exc_t, expcb), (tri_t, trimask), (ones_t, onesm)):
                nc.sync.dma_start(t[:], d[:])
            for s in (2, 3):
                nc.sync.dma_start(xt_ts[s][0][:], xtb[:, 2 * s * HSW:(2 * s + 1) * HSW])
                nc.sync.dma_start(xt_ts[s][1][:], xtb[:, (2 * s + 1) * HSW:(2 * s + 2) * HSW])
            nc.sync.dma_start(wo_t[:], wob[:])

            # ---- PE warmup: the HAM clock gate needs ~3.4us of sustained
            # matmul activity to lift PE from 1.2 to 2.4 GHz. The first real
            # matmul can only start once xt0a+wq land (~15us); run throwaway
            # matmuls on a zeroed tile during the DMA window so the real work
            # starts at full clock.
            warm_src = ep.tile([128, SW], FP16, tag="warm", bufs=1)
            nc.vector.memset(warm_src[:], 0.0)
            warm_ps = pst.tile([128, 2, SW], F32, tag="st")
            for i in range(55):
                nc.tensor.matmul(warm_ps[:, 0, :], warm_src[:, 0:128],
                                 warm_src[:], start=True, stop=True)

            # ---- norm + rope epilogue, split in two so the ssq matmul never
            # stalls PE: partA (engine-only drains of the PSUM acc) is emitted
            # right after the projection; chainB one projection target later.
            def epi_partA(acc):
                sq = ep.tile([128, SW], FP16, tag="sq")
                nc.scalar.activation(sq[:], acc[:], _ACT.Square)
                qc = ep.tile([128, SW], F32, tag="qc")
                nc.vector.tensor_copy(qc[:], acc[:])
                return sq, qc

            def epi_chainB(sq, qc, wnorm, dst, sl):
                ssq = pst.tile([128, SW], F32, tag="st")
                nc.tensor.matmul(ssq[:], ones_t[:], sq[:], start=True, stop=True)
                # rstd = (ms)^-1/2 = exp(-0.5*ln(ssq/HD + eps)); Ln/Exp live in
                # one ACT table set (sqrt does not, and DVE reciprocal is slow)
                lnv = ep.tile([128, SW], F32, tag="lnv")
                nc.scalar.activation(lnv[:], ssq[:], _ACT.Ln,
                                     scale=1.0 / HD, bias=eps_t[:])
                rstd = ep.tile([128, SW], F32, tag="rstd")
                nc.scalar.activation(rstd[:], lnv[:], _ACT.Exp, scale=-0.5)
                qn = ep.tile([128, SW], F32, tag="qn")
                nc.vector.scalar_tensor_tensor(
                    out=qn[:], in0=qc[:], scalar=wnorm[:], in1=rstd[:],
                    op0=_ALU.mult, op1=_ALU.mult)
                t1 = ep.tile([128, SW], F32, tag="t1")
                nc.vector.tensor_tensor(out=t1[:], in0=qn[:], in1=cos_t[:, sl],
                                        op=_ALU.mult)
                u = ep.tile([128, SW], F32, tag="u")
                nc.vector.tensor_tensor(out=u[0:64, :], in0=qn[64:128, :],
                                        in1=sin_t[64:128, sl], op=_ALU.mult)
                nc.vector.tensor_tensor(out=u[64:128, :], in0=qn[0:64, :],
                                        in1=sin_t[0:64, sl], op=_ALU.mult)
                nc.vector.tensor_tensor(out=dst[:, sl], in0=t1[:], in1=u[:],
                                        op=_ALU.add)

            # ---- phase 1: projections ------------------------------------
            # epi chainB's trail TWO projection targets behind, so the ssq
            # matmul never waits on the ACT-engine Square latency.
            pending = []

            def push_chain(item=None):
                if len(pending) > (1 if item is not None else 0):
                    epi_chainB(*pending.pop(0))
                if item is not None:
                    pending.append(item)

            def phase_a(s):
                sl = bass.ts(s, SW)
                _mark(nc, f"A{s}")
                xta, xtb_ = xt_ts[s]
                xsel = lambda h: (xta if h < NHT // 2 else xtb_)[:, h % (NHT // 2), :]
                targets = (
                    (lambda h: wq_t[:, h, 0:HD], wqn_t, qt0),
                    (lambda h: wq_t[:, h, HD:2 * HD], wqn_t, qt1),
                    (lambda h: wk_t[:, h, :], wkn_t, kt_sb),
                )
                for wsel, wnorm, dst in targets:
                    acc = pacc.tile([128, SW], F32, tag="acc")
                    for h in range(NHT):
                        nc.tensor.matmul(acc[:], wsel(h), xsel(h),
                                         start=(h == 0), stop=(h == NHT - 1))
                    pa = epi_partA(acc)
                    push_chain((pa[0], pa[1], wnorm, dst, sl))
                # V: accumulate directly in [seq, hd] (stationary = X^T chunk)
                for j in range(4):
                    vacc = pacc.tile([128, HD], F32, tag="acc")
                    for h in range(NHT):
                        xh = (xta if h < NHT // 2 else xtb_)
                        nc.tensor.matmul(vacc[:], xh[:, h % (NHT // 2), bass.ts(j, 128)],
                                         wv_t[:, h, :],
                                         start=(h == 0), stop=(h == NHT - 1))
                    nc.vector.tensor_copy(v_sb[:, bass.ts(4 * s + j, 128)],
                                          vacc[:])

            # ---- phase 2: attention, software-pipelined in PAIRS of key
            # tiles. Scores for a pair land in one 2-bank PSUM tile so a
            # single wide ACT exp covers both tiles ((N+352) fixed overhead
            # amortizes: 574 ns/tile vs 720, below the 645 ns/tile PE rate).
            def attention(qt, ot, s):
                sl = bass.ts(s, SW)
                nk = 4 * s + 4
                pv = ppv.tile([128, SW], F32, tag="pv")
                den = ppv.tile([128, SW], F32, tag="pv")
                exs = {}

                def emit_pair(pi):
                    t0 = 2 * pi
                    stp = pst.tile([128, 2, SW], F32, tag="st")
                    exp_ = xep.tile([128, 2, SW], FP16, tag="ex")
                    vss = []
                    for half, t in enumerate((t0, t0 + 1)):
                        off = t - 4 * s
                        vs = 0 if off < 0 else 128 * off
                        vss.append(vs)
                        nc.tensor.matmul(stp[:, half, vs:],
                                         kt_sb[:, bass.ts(t, 128)],
                                         qt[:, SW * s + vs:SW * (s + 1)],
                                         start=True, stop=True)
                    if t0 + 1 < 4 * s:
                        # both halves full: one wide exp
                        nc.scalar.activation(exp_[:, :, :], stp[:, :, :],
                                             _ACT.Exp, scale=ISQ,
                                             bias=exc_t[:])
                    else:
                        for half, vs in enumerate(vss):
                            nc.scalar.activation(exp_[:, half, vs:],
                                                 stp[:, half, vs:], _ACT.Exp,
                                                 scale=ISQ, bias=exc_t[:])
                            # diagonal 128x128 block needs the triangular mask
                            nc.vector.tensor_tensor(
                                out=exp_[:, half, vs:vs + 128],
                                in0=exp_[:, half, vs:vs + 128],
                                in1=tri_t[:], op=_ALU.mult)
                    exs[pi] = (exp_, vss)

                emit_pair(0)
                for pi in range(nk // 2):
                    if pi + 1 < nk // 2:
                        emit_pair(pi + 1)
                    exp_, vss = exs.pop(pi)
                    for half, vs in enumerate(vss):
                        t = 2 * pi + half
                        st_, sp_ = (t == 0), (t == nk - 1)
                        nc.tensor.matmul(pv[:, vs:], v_sb[:, bass.ts(t, 128)],
                                         exp_[:, half, vs:], start=st_, stop=sp_)
                        nc.tensor.matmul(den[:, vs:], ones_t[:],
                                         exp_[:, half, vs:], start=st_, stop=sp_)
                # 1/den = exp(-ln(den)) on ACT (same table set as the exps)
                lnd = ep.tile([128, SW], F32, tag="lnd")
                nc.scalar.activation(lnd[:], den[:], _ACT.Ln)
                rdn = ep.tile([128, SW], F32, tag="rdn")
                nc.scalar.activation(rdn[:], lnd[:], _ACT.Exp, scale=-1.0)
                nc.vector.tensor_tensor(out=ot[:, sl], in0=pv[:], in1=rdn[:],
                                        op=_ALU.mult)

            # ---- phase 3: output projection (per 128-row tile) ------------
            def outproj(cs):
                _mark(nc, f"C{cs}")
                for m in range(4 * cs, 4 * cs + 4):
                    ob = obp.tile([128, HID], FP16, tag="ob")
                    for np_ in range(2):
                        oup = pst.tile([128, 2, SW], F32, tag="st")
                        for half in range(2):
                            n = 2 * np_ + half
                            nc.tensor.matmul(oup[:, half, :],
                                             ot0[:, bass.ts(m, 128)],
                                             wo_t[:, 0, bass.ts(n, SW)],
                                             start=True, stop=False)
                            nc.tensor.matmul(oup[:, half, :],
                                             ot1[:, bass.ts(m, 128)],
                                             wo_t[:, 1, bass.ts(n, SW)],
                                             start=False, stop=True)
                        # single wide DVE copy per pair; ACT stays free for
                        # the interleaved attention exps
                        nc.vector.tensor_copy(ob[:, bass.ts(np_, 2 * SW)],
                                              oup[:, :, :])
                    nc.sync.dma_start(out[bass.ts(m, 128), :], ob[:])

            # schedule: A0 A1 B(s0) A2 B(s1) C0 A3 B(s2) C1 B(s3) C2 C3
            # (attention for strip s slots in right after A(s+1), keeping PE
            # dense across phase boundaries and the HAM clock warm)
            def phase_b(s):
                _mark(nc, f"B0s{s}")
                attention(qt0, ot0, s)
                _mark(nc, f"B1s{s}")
                attention(qt1, ot1, s)

            phase_a(0)
            phase_a(1)
            phase_b(0)
            phase_a(2)
            phase_b(1)
            outproj(0)
            phase_a(3)
            push_chain()
            push_chain()
            phase_b(2)
            outproj(1)
            phase_b(3)
            outproj(2)
            outproj(3)

    if legalize:
        legalize_waits(nc)
    return nc


# ---------------------------------------------------------------------------
# Host-side input prep.
def _rope_tables(position_ids: np.ndarray):
    pos = position_ids.reshape(-1).astype(np.float64)  # [S]
    j = np.arange(0, HD, 2, dtype=np.float64)
    inv_freq = 1.0 / (THETA ** (j / HD))               # [HD/2]
    freqs = np.outer(inv_freq, pos)                    # [HD/2, S]
    cos_h = np.cos(freqs)
    sin_h = np.sin(freqs)
    cosT = np.concatenate([cos_h, cos_h], axis=0).astype(np.float32)
    sinN = np.concatenate([sin_h, -sin_h], axis=0).astype(np.float32)
    return np.ascontiguousarray(cosT), np.ascontiguousarray(sinN)


def _prep_in_maps(hidden_states, Wq, Wk, Wv, Wo, q_norm_w, k_norm_w,
                  position_ids):
    X = np.asarray(hidden_states, dtype=np.float32).reshape(S, HID)
    # xtb[p, s, h, w] = X[s*SW+w, h*128+p]
    xtb = np.ascontiguousarray(
        X.reshape(NSTRIP, SW, NHT, 128).transpose(3, 0, 2, 1)
        .reshape(128, NSTRIP * NHT * SW)).astype(np.float16)
    cosT, sinN = _rope_tables(np.asarray(position_ids))
    wqn = np.ascontiguousarray(
        np.asarray(q_norm_w, dtype=np.float32).reshape(HD, 1))
    wkn = np.ascontiguousarray(
        np.asarray(k_norm_w, dtype=np.float32).reshape(HD, 1))
    kp, qp = np.meshgrid(np.arange(128), np.arange(128), indexing="ij")
    trimask = (qp >= kp).astype(np.float16)
    onesm = np.ones((128, 128), np.float16)

    Wq = np.asarray(Wq, dtype=np.float32)
    Wk = np.asarray(Wk, dtype=np.float32)
    Wv = np.asarray(Wv, dtype=np.float32)
    Wo = np.asarray(Wo, dtype=np.float32)

    in_maps = []
    for c in range(NCORES):
        kv = c // (NCORES // NKV)
        wq_c = Wq[:, c * HPC * HD:(c + 1) * HPC * HD]
        wq_l = np.ascontiguousarray(
            wq_c.reshape(NHT, 128, HPC * HD).transpose(1, 0, 2).reshape(
                128, NHT * HPC * HD)).astype(np.float16)
        wk_c = Wk[:, kv * HD:(kv + 1) * HD]
        wk_l = np.ascontiguousarray(
            wk_c.reshape(NHT, 128, HD).transpose(1, 0, 2).reshape(
                128, NHT * HD)).astype(np.float16)
        wv_c = Wv[:, kv * HD:(kv + 1) * HD]
        wv_l = np.ascontiguousarray(
            wv_c.reshape(NHT, 128, HD).transpose(1, 0, 2).reshape(
                128, NHT * HD)).astype(np.float16)
        wo_c = Wo[c * HPC * HD:(c + 1) * HPC * HD, :]
        wo_l = np.ascontiguousarray(
            wo_c.reshape(HPC, HD, HID).transpose(1, 0, 2).reshape(
                128, HPC * HID)).astype(np.float16)
        in_maps.append({
            "xtb": xtb, "wqb": wq_l, "wkb": wk_l, "wvb": wv_l, "wob": wo_l,
            "cosT": cosT, "sinN": sinN, "wqn": wqn, "wkn": wkn,
            "trimask": trimask, "onesm": onesm,
            "epsb": np.full((HD, 1), EPS, np.float32),
            "expcb": np.full((HD, 1), -EXPC, np.float32),
        })
    return in_maps


# ---------------------------------------------------------------------------
# Runner: persistent jitted shard_map over 8 cores (no donation so device
# buffers are reusable across timing iterations).
_CACHE: dict = {}


def _make_runner(nc):
    import jax
    from jax.sharding import Mesh, PartitionSpec
    try:
        from jax.experimental.shard_map import shard_map
    except ImportError:
        from jax.shard_map import shard_map
    from concourse.bass2jax import (_bass_exec_p, install_neuronx_cc_hook,
                                    partition_id_tensor)

    install_neuronx_cc_hook()

    partition_name = (nc.partition_id_tensor.name
                      if nc.partition_id_tensor else None)
    in_names, out_names, out_avals, zero_outs = [], [], [], []
    for alloc in nc.m.functions[0].allocations:
        if not isinstance(alloc, mybir.MemoryLocationSet):
            continue
        name = alloc.memorylocations[0].name
        if alloc.kind == "ExternalInput":
            if name != partition_name:
                in_names.append(name)
        elif alloc.kind == "ExternalOutput":
            shape = list(alloc.tensor_shape)
            npdt = mybir.dt.np(alloc.dtype)
            out_names.append(name)
            out_avals.append(jax.core.ShapedArray(shape, npdt))
            zero_outs.append(np.zeros(shape, npdt))

    n_params = len(in_names)
    all_in_names = list(in_names) + list(out_names)
    if partition_name is not None:
        all_in_names.append(partition_name)

    def _body(*args):
        operands = list(args)
        if partition_name is not None:
            operands.append(partition_id_tensor())
        outs = _bass_exec_p.bind(
            *operands,
            out_avals=tuple(out_avals),
            in_names=tuple(all_in_names),
            out_names=tuple(out_names),
            lowering_input_output_aliases=(),
            sim_require_finite=True,
            sim_require_nnan=True,
            nc=nc,
        )
        return tuple(outs)

    devices = jax.devices()[:NCORES]
    mesh = Mesh(np.asarray(devices), ("core",))
    n_outs = len(out_names)
    sharded = jax.jit(
        shard_map(_body, mesh=mesh,
                  in_specs=(PartitionSpec("core"),) * (n_params + n_outs),
                  out_specs=(PartitionSpec("core"),) * n_outs,
                  check_rep=False),
        keep_unused=True,
    )
    return {
        "fn": sharded, "in_names": in_names, "out_names": out_names,
        "out_avals": out_avals, "zero_outs": zero_outs, "jax": jax,
    }


def _get_runner(which="main"):
    key = f"runner_{which}"
    if key not in _CACHE:
        nc = build_nc() if which == "main" else build_null_nc()
        _CACHE[f"nc_{which}"] = nc
        _CACHE[key] = _make_runner(nc)
    return _CACHE[key]


def _device_args(in_maps, which="main"):
    r = _get_runner(which)
    jax = r["jax"]
    concat_in = [
        np.concatenate([np.asarray(in_maps[c][name]) for c in range(NCORES)],
                       axis=0)
        for name in r["in_names"]
    ]
    concat_zeros = [
        np.zeros((NCORES * z.shape[0], *z.shape[1:]), z.dtype)
        for z in r["zero_outs"]
    ]
    return [jax.device_put(a) for a in (concat_in + concat_zeros)]


def _run(dargs, which="main"):
    r = _get_runner(which)
    outs = r["fn"](*dargs)
    return outs


def kernel(**inputs) -> np.ndarray:
    in_maps = _prep_in_maps(**inputs)
    dargs = _device_args(in_maps)
    outs = _run(dargs)
    out_c = np.asarray(outs[0]).reshape(NCORES, S, HID)
    full = out_c.astype(np.float32).sum(axis=0).astype(np.float32)
    return full.reshape(B, S, HID)


def build_null_nc(legalize=True):
    """Input-identical null kernel: same ExternalInput/Output set, but only a
    trivial copy. Used to calibrate away per-dispatch input-staging overhead
    when estimating device execution time."""
    nc = bass.Bass()
    tensors = [
        ("xtb", [128, NSTRIP * NHT * SW], FP16),
        ("wqb", [128, NHT * HPC * HD], FP16),
        ("wkb", [128, NHT * HD], FP16), ("wvb", [128, NHT * HD], FP16),
        ("wob", [128, HPC * HID], FP16), ("cosT", [HD, S], F32),
        ("sinN", [HD, S], F32), ("wqn", [HD, 1], F32), ("wkn", [HD, 1], F32),
        ("trimask", [128, 128], FP16), ("onesm", [128, 128], FP16),
        ("epsb", [HD, 1], F32), ("expcb", [HD, 1], F32),
    ]
    handles = {}
    for name, shape, dt in tensors:
        handles[name] = nc.dram_tensor(name, shape, dt, kind="ExternalInput")
    out = nc.dram_tensor("out", [S, HID], FP16, kind="ExternalOutput")
    with tile.TileContext(nc) as tc:
        with tc.tile_pool(name="sb", bufs=1) as sb:
            t = sb.tile([128, 128], FP16)
            nc.sync.dma_start(t[:], handles["trimask"][:])
            nc.sync.dma_start(out[0:128, 0:128], t[:])
    if legalize:
        legalize_waits(nc)
    return nc


def timed_run(inputs, iters=60):
    """Estimate on-device execution time.

    Per-call wall time through the axon tunnel is dominated by input staging
    (~30 ms for this input set), so we interleave single calls of the real
    kernel and an input-identical null kernel and difference the medians of
    the paired per-call times."""
    import time
    in_maps = _prep_in_maps(**inputs)
    d_main = _device_args(in_maps, "main")
    d_null = _device_args(in_maps, "null")
    r_main = _get_runner("main")
    r_null = _get_runner("null")
    jax = r_main["jax"]
    jax.block_until_ready(_run(d_main, "main"))
    jax.block_until_ready(_run(d_null, "null"))

    tm, tn = [], []
    for _ in range(iters):
        t0 = time.perf_counter()
        jax.block_until_ready(_run(d_null, "null"))
        tn.append(time.perf_counter() - t0)
        t0 = time.perf_counter()
        jax.block_until_ready(_run(d_main, "main"))
        tm.append(time.perf_counter() - t0)
    tm, tn = np.array(tm), np.array(tn)
    est = float(np.median(tm) - np.median(tn))
    return max(est, 0.0), float(np.median(tm)), float(np.median(tn))
